# revision 1
# baseline (speedup 1.0000x reference)
"""Trainium2 Bass kernel for nn_Encoding_layer (highway stack + pairwise MLP
attention + fuse gates).

Sharding: data-parallel over batch B=16 across 8 NeuronCores (2 batches per
core); all dense weights replicated. No collectives.

Per-core layouts (n = 2 batches x L=1024 = 2048 token-columns):
  xTh/x1T/x2T/w3x/attT : [128, 4, 2048] bf16, "transposed" activations
                         [u mod 128, u div 128, n]
  xO                   : [128, 16, 512] bf16, row-major highway output
                         [row mod 128, row div 128, u]
  Attention: S^T[j,i] = s3[j,i] (PE, w3*x^T as lhsT) + s2[j] (ACT exp bias).
  The per-column term s1[i]+ab never enters the matmuls: exp(S+s1+ab) =
  exp(s1+ab)*exp(S), and a per-column factor cancels in the softmax, so
  relu becomes a clamp against th[i] = exp(-(s1[i]+ab)):
      M^T = max(exp(s3+s2), th)  ==  exp(relu(S_full)) / exp(s1+ab)
  Numerator att^T (lhsT = row-major x) and denominator r (lhsT = ones
  column) come from matmuls against M^T; normalization multiplies by the
  broadcast fast-approx reciprocal of r.  Broadcasts of [1,512] rows are
  PE outer-products (ones_row as lhsT) + scalar-engine copies - gpsimd
  partition_broadcast triggers multi-us ucode LIBRARY_RELOAD stalls.
"""

import numpy as np

B, L, U, H = 16, 1024, 512, 2
NCORES = 8
BPC = B // NCORES          # batches per core
N = BPC * L                # token columns per core
KU = U // 128              # 4  u-tiles
NT = N // 128              # 16 row-tiles per core
NS = N // 512              # 4  512-wide column slices per core
JT = L // 128              # 8  j-tiles per batch
IH = L // 512              # 2  i-halves per batch


def build_nc():
    import concourse.bacc as bacc
    import concourse.tile as tile
    from concourse import mybir
    from concourse.masks import make_identity

    F32 = mybir.dt.float32
    BF16 = mybir.dt.bfloat16
    AF = mybir.ActivationFunctionType
    OP = mybir.AluOpType

    nc = bacc.Bacc("TRN2", target_bir_lowering=False, debug=False,
                   num_devices=NCORES)

    x_in = nc.dram_tensor("inputs", [BPC, L, U], F32, kind="ExternalInput").ap()
    tW = nc.dram_tensor("tW", [H, U, U], F32, kind="ExternalInput").ap()
    tb = nc.dram_tensor("tb", [H, U], F32, kind="ExternalInput").ap()
    cW = nc.dram_tensor("cW", [H, U, U], F32, kind="ExternalInput").ap()
    cb = nc.dram_tensor("cb", [H, U], F32, kind="ExternalInput").ap()
    aW = nc.dram_tensor("aW", [3 * U], F32, kind="ExternalInput").ap()
    ab = nc.dram_tensor("ab", [1], F32, kind="ExternalInput").ap()
    frW = nc.dram_tensor("frW", [2 * U, U], F32, kind="ExternalInput").ap()
    frb = nc.dram_tensor("frb", [U], F32, kind="ExternalInput").ap()
    ffW = nc.dram_tensor("ffW", [2 * U, U], F32, kind="ExternalInput").ap()
    ffb = nc.dram_tensor("ffb", [U], F32, kind="ExternalInput").ap()
    out = nc.dram_tensor("out", [BPC, L, U], F32, kind="ExternalOutput").ap()

    xv = x_in.flatten_outer_dims().rearrange("(t p) u -> t p u", p=128)
    outv = out.flatten_outer_dims().rearrange("(t p) u -> t p u", p=128)

    with tile.TileContext(nc) as tc:
        with tc.tile_pool(name="pers", bufs=1) as pers:
            # ---- persistent SBUF tensors ----
            xTh = pers.tile([128, KU, N], BF16, tag="xTh")    # inputs^T
            x1T = pers.tile([128, KU, N], BF16, tag="x1T")
            x2T = pers.tile([128, KU, N], BF16, tag="x2T")
            w3x = pers.tile([128, KU, N], BF16, tag="w3x")
            attT = pers.tile([128, KU, N], BF16, tag="attT")
            xO = pers.tile([128, NT, U], BF16, tag="xO")
            tWh = pers.tile([128, H, KU, U], BF16, tag="tWh")
            cWh = pers.tile([128, H, KU, U], BF16, tag="cWh")
            ffWh = pers.tile([128, 2 * KU, U], BF16, tag="ffWh")
            frWh = pers.tile([128, 2 * KU, U], BF16, tag="frWh")
            tbsb = pers.tile([128, H, KU], F32, tag="tbsb")
            cbsb = pers.tile([128, H, KU], F32, tag="cbsb")
            awsb = pers.tile([128, 12], F32, tag="awsb")      # w1|w2|w3 cols
            w1h = pers.tile([128, KU], BF16, tag="w1h")
            w2h = pers.tile([128, KU], BF16, tag="w2h")
            ab_sb = pers.tile([1, 1], F32, tag="ab_sb")
            nab_sb = pers.tile([1, 1], F32, tag="nab_sb")
            ffb_h = pers.tile([1, U], BF16, tag="ffb_h")
            frb_h = pers.tile([1, U], BF16, tag="frb_h")
            thr = pers.tile([1, N], BF16, tag="thr")   # exp(-(s1+ab))
            s2f = pers.tile([128, NT], F32, tag="s2f")
            ones_row = pers.tile([1, 128], BF16, tag="ones_row")
            ones_col = pers.tile([128, 1], BF16, tag="ones_col")
            ident = pers.tile([128, 128], BF16, tag="ident")
            identf = pers.tile([128, 128], F32, tag="identf")

            nc.vector.memset(ones_row, 1.0)
            nc.vector.memset(ones_col, 1.0)
            make_identity(nc, ident)
            make_identity(nc, identf)

            # ================= Phase A: loads, casts, input transpose ======
            with tc.tile_pool(name="stg", bufs=8) as stg, \
                 tc.tile_pool(name="stgw", bufs=8) as stgw, \
                 tc.tile_pool(name="ptA", bufs=1, space="PSUM") as ptA:
                # inputs^T via PE transpose (PE is idle here), with
                # highway-weight loads interleaved after tg0/tg1 so layer-0
                # can start as soon as the first column group lands
                def emit_weights(l, wi):
                    wsrc, wdst = ((tW, tWh), (cW, cWh))[wi]
                    wv = wsrc[l].rearrange("(k p) m -> k p m", p=128)
                    for k in range(KU):
                        ws = stgw.tile([128, U], F32, tag="ws",
                                       name=f"ws_{l}_{wi}_{k}")
                        nc.sync.dma_start(ws, wv[k])
                        if k % 2 == 0:
                            nc.vector.tensor_copy(wdst[:, l, k, :], ws)
                        else:
                            nc.scalar.copy(wdst[:, l, k, :], ws)

                # warm the PE HAM clock-gate during the initial DMA wait:
                # ~40 tiny matmuls lift the PE to 2.4 GHz before the fp32
                # transposes (which never count as HAM-busy) begin
                warm = [ptA.tile([128, 512], F32, tag=f"ptk{k}",
                                 name=f"warm_{k}") for k in range(KU)]
                for i in range(56):
                    nc.tensor.matmul(warm[i % KU][:, 0:128], ident, ident,
                                     start=True, stop=True)
                for tg in range(NS):
                    ptk = [ptA.tile([128, 512], F32, tag=f"ptk{k}",
                                    name=f"ptk_{tg}_{k}")
                           for k in range(KU)]
                    for tt in range(4):
                        t = tg * 4 + tt
                        xs = stg.tile([128, U], F32, tag="xs",
                                      name=f"xs_{t}")
                        nc.sync.dma_start(xs, xv[t])
                        for k in range(KU):
                            nc.tensor.transpose(
                                ptk[k][:, tt * 128:(tt + 1) * 128],
                                xs[:, k * 128:(k + 1) * 128], identf)
                    for k in range(KU):
                        if k % 2 == 0:
                            nc.vector.tensor_copy(
                                xTh[:, k, tg * 512:(tg + 1) * 512], ptk[k])
                        else:
                            nc.scalar.copy(
                                xTh[:, k, tg * 512:(tg + 1) * 512], ptk[k])
                    if tg < H:
                        emit_weights(0, tg)
                    elif tg == H:
                        nc.sync.dma_start(
                            tbsb, tb.rearrange("l (m p) -> p l m", p=128))
                        nc.sync.dma_start(
                            cbsb, cb.rearrange("l (m p) -> p l m", p=128))
                        nc.sync.dma_start(
                            awsb, aW.rearrange("(w m p) -> p (w m)",
                                               p=128, w=3))
                        nc.vector.tensor_copy(w1h, awsb[:, 0:KU])
                        nc.vector.tensor_copy(w2h, awsb[:, KU:2 * KU])
                        nc.sync.dma_start(ab_sb, ab[None, :])
                        nc.scalar.mul(nab_sb, ab_sb, -1.0)
                        fb = stg.tile([1, U], F32, tag="fb")
                        nc.sync.dma_start(fb, ffb[None, :])
                        nc.vector.tensor_copy(ffb_h, fb)
                        fb2 = stg.tile([1, U], F32, tag="fb")
                        nc.sync.dma_start(fb2, frb[None, :])
                        nc.vector.tensor_copy(frb_h, fb2)
                    else:
                        emit_weights(1, 0)
                        emit_weights(1, 1)

            # ============= Phase B: highway stack (2 layers) ===========
            with tc.tile_pool(name="hwp", bufs=2, space="PSUM") as hwp, \
                 tc.tile_pool(name="hws", bufs=3) as hws:
                for l in range(H):
                    xin = xTh if l == 0 else x1T
                    xout = x1T if l == 0 else x2T
                    for t in range(NS):
                        nsl = slice(t * 512, (t + 1) * 512)
                        for m in range(KU):
                            pt = hwp.tile([128, 512], F32, tag="pt")
                            pc = hwp.tile([128, 512], F32, tag="pc")
                            for k in range(KU):
                                nc.tensor.matmul(
                                    pt, tWh[:, l, k, m * 128:(m + 1) * 128],
                                    xin[:, k, nsl],
                                    start=(k == 0), stop=(k == KU - 1))
                            for k in range(KU):
                                nc.tensor.matmul(
                                    pc, cWh[:, l, k, m * 128:(m + 1) * 128],
                                    xin[:, k, nsl],
                                    start=(k == 0), stop=(k == KU - 1))
                            th = hws.tile([128, 512], BF16, tag="th")
                            ch = hws.tile([128, 512], BF16, tag="ch")
                            nc.scalar.activation(
                                th, pt, AF.Relu, bias=tbsb[:, l, m:m + 1])
                            nc.scalar.activation(
                                ch, pc, AF.Sigmoid, bias=cbsb[:, l, m:m + 1])
                            dh = hws.tile([128, 512], BF16, tag="dh")
                            nc.vector.tensor_tensor(
                                dh, th, xin[:, m, nsl], op=OP.subtract)
                            mh = hws.tile([128, 512], BF16, tag="mh")
                            nc.vector.tensor_tensor(
                                mh, ch, dh, op=OP.mult)
                            nc.gpsimd.tensor_tensor(
                                xout[:, m, nsl], xin[:, m, nsl], mh,
                                op=OP.add)

            # ============= Phase C: attention prep =========================
            with tc.tile_pool(name="pcp", bufs=2, space="PSUM") as pcp, \
                 tc.tile_pool(name="pcp1", bufs=1, space="PSUM") as pcp1:
                # w3 * x^T  (w3 is per-partition here)
                for k in range(KU):
                    nc.vector.tensor_scalar_mul(
                        w3x[:, k, :], x2T[:, k, :], awsb[:, 8 + k:9 + k])
                # x back to row-major via PE transpose (bf16, psum staging)
                for jt in range(NT):
                    ptr = pcp.tile([128, 512], BF16, tag="ptr")
                    for k in range(KU):
                        nc.tensor.transpose(
                            ptr[:, k * 128:(k + 1) * 128],
                            x2T[:, k, jt * 128:(jt + 1) * 128], ident)
                    nc.vector.tensor_copy(xO[:, jt, :], ptr)
                # clamp threshold exp(-(s1+ab)); the factor exp(s1+ab)
                # cancels in the softmax so it never enters the matmuls
                for t in range(NS):
                    ps1 = pcp1.tile([1, 512], F32, tag="ps1")
                    for k in range(KU):
                        nc.tensor.matmul(ps1, w1h[:, k:k + 1],
                                         x2T[:, k, t * 512:(t + 1) * 512],
                                         start=(k == 0), stop=(k == KU - 1))
                    nc.scalar.activation(
                        thr[:, t * 512:(t + 1) * 512], ps1, AF.Exp,
                        bias=nab_sb, scale=-1.0)
                # s2 = x @ w2: all 16 j-tiles into one psum bank, one copy
                s2p = pcp1.tile([128, NT], F32, tag="s2p")
                for jt in range(NT):
                    for k in range(KU):
                        nc.tensor.matmul(s2p[:, jt:jt + 1],
                                         x2T[:, k, jt * 128:(jt + 1) * 128],
                                         w2h[:, k:k + 1],
                                         start=(k == 0), stop=(k == KU - 1))
                nc.vector.tensor_copy(s2f, s2p)

            # ============= Phase D: pairwise softmax attention =============
            fWv = ffW.rearrange("(k p) m -> k p m", p=128)
            rWv = frW.rearrange("(k p) m -> k p m", p=128)
            fuse_chunks = [(fWv, ffWh, k) for k in range(2 * KU)] + \
                          [(rWv, frWh, k) for k in range(2 * KU)]
            with tc.tile_pool(name="pdn", bufs=4, space="PSUM") as pdn, \
                 tc.tile_pool(name="pds", bufs=2, space="PSUM") as pds, \
                 tc.tile_pool(name="pdr", bufs=1, space="PSUM") as pdr, \
                 tc.tile_pool(name="pbc", bufs=1, space="PSUM") as pbc, \
                 tc.tile_pool(name="stgf", bufs=4) as stgf, \
                 tc.tile_pool(name="dsb", bufs=4) as dsb:
                for b in range(BPC):
                    for h in range(IH):
                        # drip-feed fuse-gate weight loads through the
                        # attention phase (gpsimd is idle here)
                        unit = b * IH + h
                        for ci in range(unit * 4, unit * 4 + 4):
                            wv_, wdst_, k_ = fuse_chunks[ci]
                            wsf = stgf.tile([128, U], F32, tag="wsf",
                                            name=f"wsf_{ci}")
                            nc.sync.dma_start(wsf, wv_[k_])
                            if ci % 2 == 0:
                                nc.vector.tensor_copy(wdst_[:, k_, :], wsf)
                            else:
                                nc.scalar.copy(wdst_[:, k_, :], wsf)
                        isl = slice(b * L + h * 512, b * L + (h + 1) * 512)
                        pn = [pdn.tile([128, 512], F32, tag="pn",
                                       name=f"pn_{b}_{h}_{du}")
                              for du in range(KU)]
                        pr = pdr.tile([1, 512], F32, tag="pr")
                        thbc = dsb.tile([128, 512], BF16, tag="thbc")
                        pb1 = pbc.tile([128, 512], F32, tag="pb",
                                       name=f"pb1_{b}_{h}")
                        nc.tensor.matmul(pb1, ones_row, thr[:, isl],
                                         start=True, stop=True)
                        nc.scalar.copy(thbc, pb1)
                        for jt in range(JT):
                            jg = b * JT + jt
                            jsl = slice(b * L + jt * 128, b * L + (jt + 1) * 128)
                            ps = pds.tile([128, 512], F32, tag="ps")
                            for k in range(KU):
                                nc.tensor.matmul(ps, w3x[:, k, jsl],
                                                 x2T[:, k, isl],
                                                 start=(k == 0),
                                                 stop=(k == KU - 1))
                            eh = dsb.tile([128, 512], BF16, tag="eh")
                            nc.scalar.activation(eh, ps, AF.Exp,
                                                 bias=s2f[:, jg:jg + 1])
                            nc.vector.tensor_tensor(eh, eh, thbc, op=OP.max)
                            for du in range(KU):
                                nc.tensor.matmul(
                                    pn[du],
                                    xO[:, jg, du * 128:(du + 1) * 128], eh,
                                    start=(jt == 0), stop=(jt == JT - 1))
                            nc.tensor.matmul(pr, ones_col, eh,
                                             start=(jt == 0),
                                             stop=(jt == JT - 1))
                        rec = dsb.tile([1, 512], F32, tag="rec")
                        nc.vector.reciprocal_approx_fast(rec, pr)
                        rech = dsb.tile([1, 512], BF16, tag="rech")
                        nc.vector.tensor_copy(rech, rec)
                        rbc = dsb.tile([128, 512], BF16, tag="rbc")
                        pb2 = pbc.tile([128, 512], F32, tag="pb",
                                       name=f"pb2_{b}_{h}")
                        nc.tensor.matmul(pb2, ones_row, rech,
                                         start=True, stop=True)
                        nc.scalar.copy(rbc, pb2)
                        # drain pn psum banks quickly via scalar, then
                        # normalize in fast bf16 on vector
                        pnh = [dsb.tile([128, 512], BF16, tag="pnh",
                                        name=f"pnh_{b}_{h}_{du}")
                               for du in range(KU)]
                        for du in range(KU):
                            if du % 2 == 0:
                                nc.scalar.copy(pnh[du], pn[du])
                            else:
                                nc.vector.tensor_copy(pnh[du], pn[du])
                        for du in range(KU):
                            nc.vector.tensor_tensor(
                                attT[:, du, isl], pnh[du], rbc, op=OP.mult)

            # ============= Phase E: fuse gates + output ====================
            with tc.tile_pool(name="pep", bufs=2, space="PSUM") as pep, \
                 tc.tile_pool(name="esb", bufs=3) as esb:
                for mt in range(NT):
                    msl = slice(mt * 128, (mt + 1) * 128)
                    x0t = esb.tile([128, U], F32, tag="x0t")
                    nc.sync.dma_start(x0t, xv[mt])
                    pz = pep.tile([128, 512], F32, tag="pz")
                    pr2 = pep.tile([128, 512], F32, tag="pr2")
                    for k in range(2 * KU):
                        lhsT = (xTh[:, k, msl] if k < KU
                                else attT[:, k - KU, msl])
                        nc.tensor.matmul(pz, lhsT, ffWh[:, k, :],
                                         start=(k == 0), stop=False)
                        nc.tensor.matmul(pr2, lhsT, frWh[:, k, :],
                                         start=(k == 0), stop=False)
                    nc.tensor.matmul(pz, ones_row, ffb_h, start=False,
                                     stop=True)
                    nc.tensor.matmul(pr2, ones_row, frb_h, start=False,
                                     stop=True)
                    zh = esb.tile([128, U], BF16, tag="zh")
                    rh = esb.tile([128, U], BF16, tag="rh")
                    q = esb.tile([128, U], F32, tag="q")
                    p2 = esb.tile([128, U], F32, tag="p2")
                    ot = esb.tile([128, U], F32, tag="ot")
                    if mt == NT - 1:
                        # last unit sets the kernel tail: shorten its
                        # serial chain by splitting across engines
                        hU = U // 2
                        nc.scalar.activation(zh, pz, AF.Sigmoid)
                        nc.scalar.square(q, zh)
                        nc.scalar.activation(rh, pr2, AF.Sigmoid)
                        nc.vector.tensor_tensor(p2[:, :hU], rh[:, :hU],
                                                x0t[:, :hU], op=OP.mult)
                        nc.gpsimd.tensor_tensor(p2[:, hU:], rh[:, hU:],
                                                x0t[:, hU:], op=OP.mult)
                        nc.vector.tensor_tensor(ot[:, :hU], q[:, :hU],
                                                p2[:, :hU], op=OP.add)
                        nc.gpsimd.tensor_tensor(ot[:, hU:], q[:, hU:],
                                                p2[:, hU:], op=OP.add)
                    else:
                        nc.scalar.activation(zh, pz, AF.Sigmoid)
                        nc.scalar.activation(rh, pr2, AF.Sigmoid)
                        nc.scalar.square(q, zh)
                        nc.vector.tensor_tensor(p2, rh, x0t, op=OP.mult)
                        nc.vector.tensor_tensor(ot, q, p2, op=OP.add)
                    nc.sync.dma_start(outv[mt], ot)

    nc.compile()
    return nc


_NC_CACHE = None


def _get_nc():
    global _NC_CACHE
    if _NC_CACHE is None:
        _NC_CACHE = build_nc()
    return _NC_CACHE


def kernel(**inputs) -> np.ndarray:
    from concourse.bass_utils import run_bass_kernel_spmd

    nc = _get_nc()
    full = {k: np.ascontiguousarray(np.asarray(v, dtype=np.float32))
            for k, v in inputs.items()}
    in_maps = []
    for c in range(NCORES):
        m = dict(full)
        m["inputs"] = np.ascontiguousarray(
            full["inputs"][c * BPC:(c + 1) * BPC])
        in_maps.append(m)
    res = run_bass_kernel_spmd(nc, in_maps, core_ids=list(range(NCORES)))
    return np.concatenate([res.results[c]["out"] for c in range(NCORES)],
                          axis=0)



# revision 5
# speedup vs baseline: 1.0391x; 1.0391x over previous
"""Trainium2 Bass kernel for nn_Encoding_layer (highway stack + pairwise MLP
attention + fuse gates).

Sharding: data-parallel over batch B=16 across 8 NeuronCores (2 batches per
core); all dense weights replicated. No collectives.

v2: fp8-e4m3 DoubleRow matmuls for the compute-heavy GEMMs.
  - DoubleRow contracts 256 rows/pass (2 fp8 weights per PE cell), ~1.4-2x
    the bf16 matmul rate.  Operand pairs are adjacent k-tiles in the free
    dim of the existing [128, KU, N] tilings, so layouts are unchanged --
    only dtypes (fp8) and 3D access patterns are new.
  - Quantization plan (numpy-validated, rel err ~2.8e-3 vs 2e-2 budget):
      highway (x fp8, W fp8*32)    scores s3 (w3x fp8*64 x x2 fp8)
      att numerator (xO fp8 x eh fp8)    att stored fp8*8
      fuse gates: x-half bf16 (x bf16, W bf16*256), att-half fp8 DR
      (att*8 x W*32); all scales undone in the activation `scale` arg.
  - exp(s)/sigmoid arguments come out of PSUM at 32x/64x/256x true scale;
    scalar.activation(func, scale=2^-k) folds the descale for free.

Per-core layouts (n = 2 batches x L=1024 = 2048 token-columns):
  xTh [128,KU,N] bf16 (inputs^T, fuse x-half lhsT); x0q8/x1q8/x2q8 fp8
  (highway act chain); w3x8, xO8 (row-major x2), attT8 fp8.
  Attention: S^T[j,i] per j-tile-PAIR: wide [128,1024] psum, per-half exp
  (bias s2[jt]) then max against thbc -> fp8 eh pair tile, which IS the
  DoubleRow moving operand [128,2,512] for the numerator.  relu-as-clamp:
  M^T = max(exp(s3+s2), exp(-(s1+ab))) (per-column factor exp(s1+ab)
  cancels in softmax).  Broadcasts of [1,512] rows are PE outer products.
"""

import numpy as np

B, L, U, H = 16, 1024, 512, 2
NCORES = 8
BPC = B // NCORES          # batches per core
N = BPC * L                # token columns per core
KU = U // 128              # 4  u-tiles
NT = N // 128              # 16 row-tiles per core
NS = N // 512              # 4  512-wide column slices per core
JT = L // 128              # 8  j-tiles per batch
IH = L // 512              # 2  i-halves per batch

WSH = 32.0                 # highway weight prescale (2^5)
WSA = 64.0                 # aW prescale (2^6)
WSF = 256.0                # fuse-gate effective prescale (2^8)
ATS = 8.0                  # att fp8 prescale (2^3)


def build_nc():
    import concourse.bacc as bacc
    import concourse.tile as tile
    from concourse import mybir
    from concourse.masks import make_identity

    F32 = mybir.dt.float32
    BF16 = mybir.dt.bfloat16
    F8 = mybir.dt.float8e4
    AF = mybir.ActivationFunctionType
    OP = mybir.AluOpType
    DR = mybir.MatmulPerfMode.DoubleRow

    nc = bacc.Bacc("TRN2", target_bir_lowering=False, debug=False,
                   num_devices=NCORES)

    x_in = nc.dram_tensor("inputs", [BPC, L, U], F32, kind="ExternalInput").ap()
    tW = nc.dram_tensor("tW", [H, U, U], F32, kind="ExternalInput").ap()
    tb = nc.dram_tensor("tb", [H, U], F32, kind="ExternalInput").ap()
    cW = nc.dram_tensor("cW", [H, U, U], F32, kind="ExternalInput").ap()
    cb = nc.dram_tensor("cb", [H, U], F32, kind="ExternalInput").ap()
    aW = nc.dram_tensor("aW", [3 * U], F32, kind="ExternalInput").ap()
    ab = nc.dram_tensor("ab", [1], F32, kind="ExternalInput").ap()
    frW = nc.dram_tensor("frW", [2 * U, U], F32, kind="ExternalInput").ap()
    frb = nc.dram_tensor("frb", [U], F32, kind="ExternalInput").ap()
    ffW = nc.dram_tensor("ffW", [2 * U, U], F32, kind="ExternalInput").ap()
    ffb = nc.dram_tensor("ffb", [U], F32, kind="ExternalInput").ap()
    out = nc.dram_tensor("out", [BPC, L, U], F32, kind="ExternalOutput").ap()

    xv = x_in.flatten_outer_dims().rearrange("(t p) u -> t p u", p=128)
    outv = out.flatten_outer_dims().rearrange("(t p) u -> t p u", p=128)

    def pair(t, k2, sl=None):
        """[128, 2, *] DoubleRow view of adjacent k-tiles k2*2, k2*2+1."""
        return t[:, 2 * k2:2 * k2 + 2, sl] if sl is not None \
            else t[:, 2 * k2:2 * k2 + 2, :]

    with tile.TileContext(nc) as tc:
        with tc.tile_pool(name="pers", bufs=1) as pers:
            # ---- persistent SBUF tensors ----
            xTh = pers.tile([128, KU, N], BF16, tag="xTh")     # inputs^T bf16
            x0q8 = pers.tile([128, KU, N], F8, tag="x0q8")     # inputs^T fp8
            x1q8 = pers.tile([128, KU, N], F8, tag="x1q8")
            x2q8 = pers.tile([128, KU, N], F8, tag="x2q8")
            w3x8 = pers.tile([128, KU, N], F8, tag="w3x8")     # (w3*64)*x2^T
            attT8 = pers.tile([128, KU, N], F8, tag="attT8")   # att^T * 8
            xO8 = pers.tile([128, NT, U], F8, tag="xO8")       # row-major x2
            tWh8 = pers.tile([128, H, KU, U], F8, tag="tWh8")  # *32
            cWh8 = pers.tile([128, H, KU, U], F8, tag="cWh8")  # *32
            ffWx = pers.tile([128, KU, U], BF16, tag="ffWx")   # x-half *256
            frWx = pers.tile([128, KU, U], BF16, tag="frWx")
            ffW8 = pers.tile([128, KU, U], F8, tag="ffW8")     # att-half *32
            frW8 = pers.tile([128, KU, U], F8, tag="frW8")
            tbsb = pers.tile([128, H, KU], F32, tag="tbsb")
            cbsb = pers.tile([128, H, KU], F32, tag="cbsb")
            awsb = pers.tile([128, 12], F32, tag="awsb")       # w1|w2|w3 cols
            w1h8 = pers.tile([128, KU, 16], F8, tag="w1h8")    # *64, col 0
            w2h8 = pers.tile([128, KU, 16], F8, tag="w2h8")    # *64, col 0
            aw3s = pers.tile([128, KU], F32, tag="aw3s")       # w3 * 64 f32
            ab_sb = pers.tile([1, 1], F32, tag="ab_sb")
            nab_sb = pers.tile([1, 1], F32, tag="nab_sb")
            ffb_h = pers.tile([1, U], BF16, tag="ffb_h")       # *256
            frb_h = pers.tile([1, U], BF16, tag="frb_h")       # *256
            thr = pers.tile([1, N], BF16, tag="thr")   # exp(-(s1+ab))
            s2f = pers.tile([128, NT], F32, tag="s2f")
            ones_row = pers.tile([1, 128], BF16, tag="ones_row")
            ones2c8 = pers.tile([128, 32], F8, tag="ones2c8")  # DR ones pairs
            ident8 = pers.tile([128, 128], F8, tag="ident8")
            identf = pers.tile([128, 128], F32, tag="identf")

            nc.vector.memset(ones_row, 1.0)
            nc.vector.memset(ones2c8, 1.0)
            make_identity(nc, ident8)
            make_identity(nc, identf)

            # ================= Phase A: loads, casts, input transpose ======
            with tc.tile_pool(name="stg", bufs=8) as stg, \
                 tc.tile_pool(name="stgw", bufs=8) as stgw, \
                 tc.tile_pool(name="ptA", bufs=1, space="PSUM") as ptA:
                # highway-weight loads interleaved after tg0/tg1 so layer-0
                # can start as soon as the first column group lands
                def emit_weights(l, wi):
                    wsrc, wdst = ((tW, tWh8), (cW, cWh8))[wi]
                    wv = wsrc[l].rearrange("(k p) m -> k p m", p=128)
                    for k in range(KU):
                        ws = stgw.tile([128, U], F32, tag="ws",
                                       name=f"ws_{l}_{wi}_{k}")
                        nc.sync.dma_start(ws, wv[k])
                        if k % 2 == 0:
                            nc.vector.tensor_scalar_mul(
                                wdst[:, l, k, :], ws, WSH)
                        else:
                            nc.scalar.mul(wdst[:, l, k, :], ws, WSH)

                # warm the PE HAM clock-gate during the initial DMA wait
                warm = [ptA.tile([128, 512], F32, tag=f"ptk{k}",
                                 name=f"warm_{k}") for k in range(KU)]
                for i in range(56):
                    nc.tensor.matmul(warm[i % KU][:, 0:128], identf,
                                     identf, start=True, stop=True)
                for tg in range(NS):
                    ptk = [ptA.tile([128, 512], F32, tag=f"ptk{k}",
                                    name=f"ptk_{tg}_{k}")
                           for k in range(KU)]
                    for tt in range(4):
                        t = tg * 4 + tt
                        xs = stg.tile([128, U], F32, tag="xs",
                                      name=f"xs_{t}")
                        nc.sync.dma_start(xs, xv[t])
                        for k in range(KU):
                            nc.tensor.transpose(
                                ptk[k][:, tt * 128:(tt + 1) * 128],
                                xs[:, k * 128:(k + 1) * 128], identf)
                    for k in range(KU):
                        sl = slice(tg * 512, (tg + 1) * 512)
                        if k % 2 == 0:
                            nc.vector.tensor_copy(xTh[:, k, sl], ptk[k])
                            nc.scalar.copy(x0q8[:, k, sl], ptk[k])
                        else:
                            nc.scalar.copy(xTh[:, k, sl], ptk[k])
                            nc.vector.tensor_copy(x0q8[:, k, sl], ptk[k])
                    # keep PE warm across the DMA-bound stretch
                    for i in range(8):
                        nc.tensor.matmul(ptk[0][:, 0:128], identf, identf,
                                         start=True, stop=True)
                    if tg < H:
                        emit_weights(0, tg)
                    elif tg == H:
                        nc.sync.dma_start(
                            tbsb, tb.rearrange("l (m p) -> p l m", p=128))
                        nc.sync.dma_start(
                            cbsb, cb.rearrange("l (m p) -> p l m", p=128))
                        nc.sync.dma_start(
                            awsb, aW.rearrange("(w m p) -> p (w m)",
                                               p=128, w=3))
                        for k in range(KU):
                            nc.vector.tensor_scalar_mul(
                                w1h8[:, k, 0:1], awsb[:, k:k + 1], WSA)
                            nc.vector.tensor_scalar_mul(
                                w2h8[:, k, 0:1], awsb[:, KU + k:KU + k + 1],
                                WSA)
                            nc.scalar.mul(aw3s[:, k:k + 1],
                                          awsb[:, 8 + k:9 + k], WSA)
                        nc.sync.dma_start(ab_sb, ab[None, :])
                        nc.scalar.mul(nab_sb, ab_sb, -1.0)
                        fb = stg.tile([1, U], F32, tag="fb")
                        nc.sync.dma_start(fb, ffb[None, :])
                        nc.vector.tensor_scalar_mul(ffb_h, fb, WSF)
                        fb2 = stg.tile([1, U], F32, tag="fb")
                        nc.sync.dma_start(fb2, frb[None, :])
                        nc.vector.tensor_scalar_mul(frb_h, fb2, WSF)
                    else:
                        emit_weights(1, 0)
                        emit_weights(1, 1)

            # ============= Phase B: highway stack (2 layers) ===========
            # wide [128,1024] 2-bank psum tiles; fp8 DoubleRow matmuls
            with tc.tile_pool(name="hwp", bufs=2, space="PSUM") as hwp, \
                 tc.tile_pool(name="hws", bufs=3) as hws:
                for l in range(H):
                    xin = x0q8 if l == 0 else x1q8
                    xout = x1q8 if l == 0 else x2q8
                    for tp in range(NS // 2):          # 1024-token slabs
                        wsl = slice(tp * 1024, (tp + 1) * 1024)
                        for m in range(KU):
                            msl = slice(m * 128, (m + 1) * 128)
                            pt = hwp.tile([128, 1024], F32, tag="pt")
                            pc = hwp.tile([128, 1024], F32, tag="pc")
                            for h2 in range(2):
                                nsl = slice(tp * 1024 + h2 * 512,
                                            tp * 1024 + (h2 + 1) * 512)
                                psl = slice(h2 * 512, (h2 + 1) * 512)
                                for kk in range(KU // 2):
                                    nc.tensor.matmul(
                                        pt[:, psl],
                                        pair(tWh8[:, l], kk, msl),
                                        pair(xin, kk, nsl), perf_mode=DR,
                                        start=(kk == 0), stop=(kk == 1))
                                for kk in range(KU // 2):
                                    nc.tensor.matmul(
                                        pc[:, psl],
                                        pair(cWh8[:, l], kk, msl),
                                        pair(xin, kk, nsl), perf_mode=DR,
                                        start=(kk == 0), stop=(kk == 1))
                            th = hws.tile([128, 1024], BF16, tag="th")
                            ch = hws.tile([128, 1024], BF16, tag="ch")
                            nc.scalar.activation(
                                th, pt, AF.Relu, bias=tbsb[:, l, m:m + 1],
                                scale=1.0 / WSH)
                            nc.scalar.activation(
                                ch, pc, AF.Sigmoid, bias=cbsb[:, l, m:m + 1],
                                scale=1.0 / WSH)
                            dh = hws.tile([128, 1024], BF16, tag="dh")
                            nc.vector.tensor_tensor(
                                dh, th, xin[:, m, wsl], op=OP.subtract)
                            mh = hws.tile([128, 1024], BF16, tag="mh")
                            nc.gpsimd.tensor_tensor(
                                mh, ch, dh, op=OP.mult)
                            nc.vector.tensor_tensor(
                                xout[:, m, wsl], xin[:, m, wsl], mh,
                                op=OP.add)

            # ============= Phase C: attention prep =========================
            with tc.tile_pool(name="pcp", bufs=2, space="PSUM") as pcp, \
                 tc.tile_pool(name="pcp1", bufs=1, space="PSUM") as pcp1:
                # (w3*64) * x2^T  (w3 per-partition)
                for k in range(KU):
                    nc.vector.tensor_scalar_mul(
                        w3x8[:, k, :], x2q8[:, k, :], aw3s[:, k:k + 1])
                # x2 back to row-major via PE transpose (fp8, psum staging;
                # fp8 transpose hw requires output element step of 2)
                for jt in range(NT):
                    ptr = pcp.tile([128, 1024], F8, tag="ptr")
                    ptv = ptr.rearrange("p (n two) -> p n two", two=2)
                    for k in range(KU):
                        nc.tensor.transpose(
                            ptv[:, k * 128:(k + 1) * 128, 0:1],
                            x2q8[:, k, jt * 128:(jt + 1) * 128], ident8)
                    if jt % 2 == 0:
                        nc.vector.tensor_copy(xO8[:, jt, :], ptv[:, :, 0:1])
                    else:
                        nc.scalar.copy(xO8[:, jt, :], ptv[:, :, 0:1])
                # clamp threshold exp(-(s1+ab)); factor cancels in softmax
                for t in range(NS):
                    ps1 = pcp1.tile([1, 512], F32, tag="ps1")
                    for kk in range(KU // 2):
                        nc.tensor.matmul(
                            ps1, pair(w1h8, kk, slice(0, 1)),
                            pair(x2q8, kk, slice(t * 512, (t + 1) * 512)),
                            perf_mode=DR,
                            start=(kk == 0), stop=(kk == 1))
                    nc.scalar.activation(
                        thr[:, t * 512:(t + 1) * 512], ps1, AF.Exp,
                        bias=nab_sb, scale=-1.0 / WSA)
                # s2 = x2 @ w2 (*64): all 16 j-tiles into one psum bank
                s2p = pcp1.tile([128, NT], F32, tag="s2p")
                for jt in range(NT):
                    jsl = slice(jt * 128, (jt + 1) * 128)
                    for kk in range(KU // 2):
                        nc.tensor.matmul(
                            s2p[:, jt:jt + 1],
                            pair(x2q8, kk, jsl),
                            pair(w2h8, kk, slice(0, 1)),
                            perf_mode=DR,
                            start=(kk == 0), stop=(kk == 1))
                nc.scalar.mul(s2f, s2p, 1.0 / WSA)

            # ============= Phase D: pairwise softmax attention =============
            fWv = ffW.rearrange("(k p) m -> k p m", p=128)
            rWv = frW.rearrange("(k p) m -> k p m", p=128)
            # x-half (k 0..3) -> bf16 *256 ; att-half (k 4..7) -> fp8 *32
            fuse_chunks = ([(fWv, ffWx, ffW8, k) for k in range(2 * KU)] +
                           [(rWv, frWx, frW8, k) for k in range(2 * KU)])
            with tc.tile_pool(name="pdn", bufs=4, space="PSUM") as pdn, \
                 tc.tile_pool(name="pds", bufs=1, space="PSUM") as pds, \
                 tc.tile_pool(name="pdr", bufs=1, space="PSUM") as pdr, \
                 tc.tile_pool(name="pbc", bufs=1, space="PSUM") as pbc, \
                 tc.tile_pool(name="stgf", bufs=4) as stgf, \
                 tc.tile_pool(name="dsb", bufs=4) as dsb, \
                 tc.tile_pool(name="ehp", bufs=2) as ehp:
                ones2v = ones2c8.rearrange("p (two s) -> p two s", two=2)
                for b in range(BPC):
                    for h in range(IH):
                        # drip-feed fuse-gate weight loads (DMA idle here)
                        unit = b * IH + h
                        for ci in range(unit * 4, unit * 4 + 4):
                            wv_, wbf_, w8_, k_ = fuse_chunks[ci]
                            wsf = stgf.tile([128, U], F32, tag="wsf",
                                            name=f"wsf_{ci}")
                            nc.sync.dma_start(wsf, wv_[k_])
                            if k_ < KU:
                                if ci % 2 == 0:
                                    nc.vector.tensor_scalar_mul(
                                        wbf_[:, k_, :], wsf, WSF)
                                else:
                                    nc.scalar.mul(wbf_[:, k_, :], wsf, WSF)
                            else:
                                if ci % 2 == 0:
                                    nc.vector.tensor_scalar_mul(
                                        w8_[:, k_ - KU, :], wsf, WSH)
                                else:
                                    nc.scalar.mul(w8_[:, k_ - KU, :], wsf,
                                                  WSH)
                        isl = slice(b * L + h * 512, b * L + (h + 1) * 512)
                        pn = [pdn.tile([128, 512], F32, tag="pn",
                                       name=f"pn_{b}_{h}_{du}")
                              for du in range(KU)]
                        pr = pdr.tile([1, 512], F32, tag="pr")
                        thbc = dsb.tile([128, 512], BF16, tag="thbc")
                        pb1 = pbc.tile([128, 512], F32, tag="pb",
                                       name=f"pb1_{b}_{h}")
                        nc.tensor.matmul(pb1, ones_row, thr[:, isl],
                                         start=True, stop=True)
                        nc.scalar.copy(thbc, pb1)
                        for p in range(JT // 2):      # j-tile pairs
                            jg = b * JT + 2 * p
                            ps = pds.tile([128, 1024], F32, tag="ps",
                                          name=f"ps_{b}_{h}_{p}")
                            ehb = ehp.tile([128, 1024], F8, tag="ehb",
                                           name=f"ehb_{b}_{h}_{p}")
                            ehbf = dsb.tile([128, 1024], BF16, tag="ehbf")
                            for half in range(2):
                                jsl = slice((jg + half) * 128,
                                            (jg + half + 1) * 128)
                                hsl = slice(half * 512, (half + 1) * 512)
                                for kk in range(KU // 2):
                                    nc.tensor.matmul(
                                        ps[:, hsl], pair(w3x8, kk, jsl),
                                        pair(x2q8, kk, isl), perf_mode=DR,
                                        start=(kk == 0), stop=(kk == 1))
                                nc.scalar.activation(
                                    ehbf[:, hsl], ps[:, hsl], AF.Exp,
                                    bias=s2f[:, jg + half:jg + half + 1],
                                    scale=1.0 / WSA)
                                nc.vector.tensor_tensor(
                                    ehb[:, hsl], ehbf[:, hsl], thbc,
                                    op=OP.max)
                            ehv = ehb.rearrange("p (two n) -> p two n",
                                                two=2)
                            for du in range(KU):
                                nc.tensor.matmul(
                                    pn[du],
                                    xO8[:, jg:jg + 2,
                                        du * 128:(du + 1) * 128],
                                    ehv, perf_mode=DR,
                                    start=(p == 0), stop=(p == JT // 2 - 1))
                            nc.tensor.matmul(
                                pr, ones2v[:, :, 0:1], ehv, perf_mode=DR,
                                start=(p == 0), stop=(p == JT // 2 - 1))
                        rec = dsb.tile([1, 512], F32, tag="rec")
                        nc.vector.reciprocal_approx_fast(rec, pr)
                        rech = dsb.tile([1, 512], BF16, tag="rech")
                        nc.scalar.mul(rech, rec, ATS)
                        rbc = dsb.tile([128, 512], BF16, tag="rbc")
                        pb2 = pbc.tile([128, 512], F32, tag="pb",
                                       name=f"pb2_{b}_{h}")
                        nc.tensor.matmul(pb2, ones_row, rech,
                                         start=True, stop=True)
                        nc.scalar.copy(rbc, pb2)
                        # drain pn psum banks, normalize (*8) into fp8 attT
                        pnh = [dsb.tile([128, 512], BF16, tag=f"pnh{du}",
                                        name=f"pnh_{b}_{h}_{du}")
                               for du in range(KU)]
                        for du in range(KU):
                            if du % 2 == 0:
                                nc.scalar.copy(pnh[du], pn[du])
                            else:
                                nc.vector.tensor_copy(pnh[du], pn[du])
                        for du in range(KU):
                            nc.vector.tensor_tensor(
                                attT8[:, du, isl], pnh[du], rbc, op=OP.mult)

            # ============= Phase E: fuse gates + output ====================
            with tc.tile_pool(name="pep", bufs=2, space="PSUM") as pep, \
                 tc.tile_pool(name="esb", bufs=3) as esb:
                for mt in range(NT):
                    msl = slice(mt * 128, (mt + 1) * 128)
                    x0t = esb.tile([128, U], F32, tag="x0t")
                    nc.sync.dma_start(x0t, xv[mt])
                    pz = pep.tile([128, 512], F32, tag="pz")
                    pr2 = pep.tile([128, 512], F32, tag="pr2")
                    for k in range(KU):          # x-half, bf16
                        nc.tensor.matmul(pz, xTh[:, k, msl], ffWx[:, k, :],
                                         start=(k == 0), stop=False)
                        nc.tensor.matmul(pr2, xTh[:, k, msl], frWx[:, k, :],
                                         start=(k == 0), stop=False)
                    for kk in range(KU // 2):    # att-half, fp8 DR
                        nc.tensor.matmul(pz, pair(attT8, kk, msl),
                                         pair(ffW8, kk), perf_mode=DR,
                                         start=False, stop=False)
                        nc.tensor.matmul(pr2, pair(attT8, kk, msl),
                                         pair(frW8, kk), perf_mode=DR,
                                         start=False, stop=False)
                    nc.tensor.matmul(pz, ones_row, ffb_h, start=False,
                                     stop=True)
                    nc.tensor.matmul(pr2, ones_row, frb_h, start=False,
                                     stop=True)
                    zh = esb.tile([128, U], BF16, tag="zh")
                    rh = esb.tile([128, U], BF16, tag="rh")
                    q = esb.tile([128, U], F32, tag="q")
                    p2 = esb.tile([128, U], F32, tag="p2")
                    ot = esb.tile([128, U], F32, tag="ot")
                    if mt == NT - 1:
                        # shorten the kernel tail: split across engines
                        hU = U // 2
                        nc.scalar.activation(zh, pz, AF.Sigmoid,
                                             scale=1.0 / WSF)
                        nc.scalar.square(q, zh)
                        nc.scalar.activation(rh, pr2, AF.Sigmoid,
                                             scale=1.0 / WSF)
                        nc.vector.tensor_tensor(p2[:, :hU], rh[:, :hU],
                                                x0t[:, :hU], op=OP.mult)
                        nc.gpsimd.tensor_tensor(p2[:, hU:], rh[:, hU:],
                                                x0t[:, hU:], op=OP.mult)
                        nc.vector.tensor_tensor(ot[:, :hU], q[:, :hU],
                                                p2[:, :hU], op=OP.add)
                        nc.gpsimd.tensor_tensor(ot[:, hU:], q[:, hU:],
                                                p2[:, hU:], op=OP.add)
                    else:
                        nc.scalar.activation(zh, pz, AF.Sigmoid,
                                             scale=1.0 / WSF)
                        nc.scalar.activation(rh, pr2, AF.Sigmoid,
                                             scale=1.0 / WSF)
                        nc.scalar.square(q, zh)
                        nc.vector.tensor_tensor(p2, rh, x0t, op=OP.mult)
                        nc.vector.tensor_tensor(ot, q, p2, op=OP.add)
                    nc.sync.dma_start(outv[mt], ot)

    nc.compile()
    return nc


_NC_CACHE = None


def _get_nc():
    global _NC_CACHE
    if _NC_CACHE is None:
        _NC_CACHE = build_nc()
    return _NC_CACHE


def kernel(**inputs) -> np.ndarray:
    from concourse.bass_utils import run_bass_kernel_spmd

    nc = _get_nc()
    full = {k: np.ascontiguousarray(np.asarray(v, dtype=np.float32))
            for k, v in inputs.items()}
    in_maps = []
    for c in range(NCORES):
        m = dict(full)
        m["inputs"] = np.ascontiguousarray(
            full["inputs"][c * BPC:(c + 1) * BPC])
        in_maps.append(m)
    res = run_bass_kernel_spmd(nc, in_maps, core_ids=list(range(NCORES)))
    return np.concatenate([res.results[c]["out"] for c in range(NCORES)],
                          axis=0)


# revision 9
# speedup vs baseline: 1.1194x; 1.0772x over previous
"""Trainium2 Bass kernel for nn_Encoding_layer (highway stack + pairwise MLP
attention + fuse gates).

Sharding: data-parallel over batch B=16 across 8 NeuronCores (2 batches per
core); all dense weights replicated. No collectives.

v3: fp8-e4m3 DoubleRow matmuls for the compute-heavy GEMMs, with the
schedule restructured to keep the PE HAM clock-gate warm:
  - DoubleRow contracts 256 rows/pass (2 fp8 weights per PE cell); operand
    pairs are adjacent k-tiles in the free dim of the [128, KU, N] tilings.
  - Quantization (numpy-validated, rel err ~3e-3 vs 2e-2 budget):
      highway (x fp8, W fp8*32)   scores s3 (w3x fp8*64 x x2 fp8)
      att numerator (xO fp8 x eh fp8)   att stored fp8*8
      fuse gates: x-half bf16 (W bf16*256) + att-half fp8 DR (att*8 x W*32)
    All scales undone via scalar.activation(func, scale=2^-k).
  - Attention prep (row-major transposes, s1/s2/thr) is emitted per-slab
    inside highway layer 2, filling PE slack in the elementwise-bound
    highway stretch.
  - Phase D per (b,h) unit is two-staged: (1) all four j-tile-pair score
    blocks -> exp -> fp8 eh pair tiles (SBUF), (2) denominator then
    du-major numerator accumulation, so wide score psum is double-buffered
    within the 8-bank budget.
  - eh pair tiles [128,2,512] fp8 are exactly the DoubleRow moving operand
    of the numerator.  relu-as-clamp: M^T = max(exp(s3+s2), exp(-(s1+ab)))
    (the per-column factor exp(s1+ab) cancels in the softmax).
"""

import numpy as np

B, L, U, H = 16, 1024, 512, 2
NCORES = 8
BPC = B // NCORES          # batches per core
N = BPC * L                # token columns per core
KU = U // 128              # 4  u-tiles
NT = N // 128              # 16 row-tiles per core
NS = N // 512              # 4  512-wide column slices per core
JT = L // 128              # 8  j-tiles per batch
IH = L // 512              # 2  i-halves per batch

WSH = 32.0                 # highway weight prescale (2^5)
WSA = 64.0                 # aW prescale (2^6)
WSF = 256.0                # fuse-gate effective prescale (2^8)
ATS = 8.0                  # att fp8 prescale (2^3)


def build_nc():
    import concourse.bacc as bacc
    import concourse.tile as tile
    from concourse import mybir
    from concourse.masks import make_identity

    F32 = mybir.dt.float32
    BF16 = mybir.dt.bfloat16
    F8 = mybir.dt.float8e4
    AF = mybir.ActivationFunctionType
    OP = mybir.AluOpType
    DR = mybir.MatmulPerfMode.DoubleRow

    nc = bacc.Bacc("TRN2", target_bir_lowering=False, debug=False,
                   num_devices=NCORES)

    x_in = nc.dram_tensor("inputs", [BPC, L, U], F32, kind="ExternalInput").ap()
    tW = nc.dram_tensor("tW", [H, U, U], F32, kind="ExternalInput").ap()
    tb = nc.dram_tensor("tb", [H, U], F32, kind="ExternalInput").ap()
    cW = nc.dram_tensor("cW", [H, U, U], F32, kind="ExternalInput").ap()
    cb = nc.dram_tensor("cb", [H, U], F32, kind="ExternalInput").ap()
    aW = nc.dram_tensor("aW", [3 * U], F32, kind="ExternalInput").ap()
    ab = nc.dram_tensor("ab", [1], F32, kind="ExternalInput").ap()
    frW = nc.dram_tensor("frW", [2 * U, U], F32, kind="ExternalInput").ap()
    frb = nc.dram_tensor("frb", [U], F32, kind="ExternalInput").ap()
    ffW = nc.dram_tensor("ffW", [2 * U, U], F32, kind="ExternalInput").ap()
    ffb = nc.dram_tensor("ffb", [U], F32, kind="ExternalInput").ap()
    out = nc.dram_tensor("out", [BPC, L, U], F32, kind="ExternalOutput").ap()

    xv = x_in.flatten_outer_dims().rearrange("(t p) u -> t p u", p=128)
    outv = out.flatten_outer_dims().rearrange("(t p) u -> t p u", p=128)

    def pair(t, k2, sl=None):
        """[128, 2, *] DoubleRow view of adjacent k-tiles k2*2, k2*2+1."""
        return t[:, 2 * k2:2 * k2 + 2, sl] if sl is not None \
            else t[:, 2 * k2:2 * k2 + 2, :]

    with tile.TileContext(nc) as tc:
        with tc.tile_pool(name="pers", bufs=1) as pers:
            # ---- persistent SBUF tensors ----
            xTh = pers.tile([128, KU, N], BF16, tag="xTh")     # inputs^T bf16
            x0q8 = pers.tile([128, KU, N], F8, tag="x0q8")     # inputs^T fp8
            x1q8 = pers.tile([128, KU, N], F8, tag="x1q8")
            x2q8 = pers.tile([128, KU, N], F8, tag="x2q8")
            w3x8 = pers.tile([128, KU, N], F8, tag="w3x8")     # (w3*64)*x2^T
            attT8 = pers.tile([128, KU, N], F8, tag="attT8")   # att^T * 8
            xO8 = pers.tile([128, NT, U], F8, tag="xO8")       # row-major x2
            tWh8 = pers.tile([128, H, KU, U], F8, tag="tWh8")  # *32
            cWh8 = pers.tile([128, H, KU, U], F8, tag="cWh8")  # *32
            ffWx = pers.tile([128, KU, U], BF16, tag="ffWx")   # x-half *256
            frWx = pers.tile([128, KU, U], BF16, tag="frWx")
            ffW8 = pers.tile([128, KU, U], F8, tag="ffW8")     # att-half *32
            frW8 = pers.tile([128, KU, U], F8, tag="frW8")
            tbsb = pers.tile([128, H, KU], F32, tag="tbsb")
            cbsb = pers.tile([128, H, KU], F32, tag="cbsb")
            awsb = pers.tile([128, 12], F32, tag="awsb")       # w1|w2|w3 cols
            w1h8 = pers.tile([128, KU, 16], F8, tag="w1h8")    # *64, col 0
            w2h8 = pers.tile([128, KU, 16], F8, tag="w2h8")    # *64, col 0
            aw3s = pers.tile([128, KU], F32, tag="aw3s")       # w3 * 64 f32
            ab_sb = pers.tile([1, 1], F32, tag="ab_sb")
            nab_sb = pers.tile([1, 1], F32, tag="nab_sb")
            ffb_h = pers.tile([1, U], BF16, tag="ffb_h")       # *256
            frb_h = pers.tile([1, U], BF16, tag="frb_h")       # *256
            thr = pers.tile([1, N], BF16, tag="thr")   # exp(-(s1+ab))
            s2f = pers.tile([128, NT], F32, tag="s2f")
            ones_row = pers.tile([1, 128], BF16, tag="ones_row")
            ones2c8 = pers.tile([128, 32], F8, tag="ones2c8")  # DR ones pairs
            identb = pers.tile([128, 128], BF16, tag="identb")
            ident8 = pers.tile([128, 128], F8, tag="ident8")
            identf = pers.tile([128, 128], F32, tag="identf")

            nc.vector.memset(ones_row, 1.0)
            nc.vector.memset(ones2c8, 1.0)
            make_identity(nc, identb)
            make_identity(nc, ident8)
            make_identity(nc, identf)

            # ================= Phase A: loads, casts, input transpose ======
            with tc.tile_pool(name="stg", bufs=8) as stg, \
                 tc.tile_pool(name="stgw", bufs=8) as stgw, \
                 tc.tile_pool(name="ptA", bufs=1, space="PSUM") as ptA:
                warmp = ptA.tile([128, 512], F32, tag="warmp")

                def keep_warm(n, who):
                    for i in range(n):
                        nc.tensor.matmul(warmp[:, 0:128], identb, identb,
                                         start=True, stop=True)

                # highway-weight loads interleaved after tg0/tg1 so layer-0
                # can start as soon as the first column group lands
                def emit_weights(l, wi):
                    wsrc, wdst = ((tW, tWh8), (cW, cWh8))[wi]
                    wv = wsrc[l].rearrange("(k p) m -> k p m", p=128)
                    for k in range(KU):
                        ws = stgw.tile([128, U], F32, tag="ws",
                                       name=f"ws_{l}_{wi}_{k}")
                        nc.sync.dma_start(ws, wv[k])
                        if k % 2 == 0:
                            nc.vector.tensor_scalar_mul(
                                wdst[:, l, k, :], ws, WSH)
                        else:
                            nc.scalar.mul(wdst[:, l, k, :], ws, WSH)

                # warm the PE HAM clock-gate during the initial DMA wait
                keep_warm(48, "init")
                for tg in range(NS):
                    ptk = [ptA.tile([128, 512], F32, tag=f"ptk{k}",
                                    name=f"ptk_{tg}_{k}")
                           for k in range(KU)]
                    for tt in range(4):
                        t = tg * 4 + tt
                        xs = stg.tile([128, U], F32, tag="xs",
                                      name=f"xs_{t}")
                        nc.sync.dma_start(xs, xv[t])
                        for k in range(KU):
                            nc.tensor.transpose(
                                ptk[k][:, tt * 128:(tt + 1) * 128],
                                xs[:, k * 128:(k + 1) * 128], identf)
                    for k in range(KU):
                        sl = slice(tg * 512, (tg + 1) * 512)
                        if k % 2 == 0:
                            nc.vector.tensor_copy(xTh[:, k, sl], ptk[k])
                            nc.scalar.copy(x0q8[:, k, sl], ptk[k])
                        else:
                            nc.scalar.copy(xTh[:, k, sl], ptk[k])
                            nc.vector.tensor_copy(x0q8[:, k, sl], ptk[k])
                    keep_warm(10, f"tg{tg}")
                    if tg < H:
                        emit_weights(0, tg)
                    elif tg == H:
                        nc.sync.dma_start(
                            tbsb, tb.rearrange("l (m p) -> p l m", p=128))
                        nc.sync.dma_start(
                            cbsb, cb.rearrange("l (m p) -> p l m", p=128))
                        nc.sync.dma_start(
                            awsb, aW.rearrange("(w m p) -> p (w m)",
                                               p=128, w=3))
                        for k in range(KU):
                            nc.vector.tensor_scalar_mul(
                                w1h8[:, k, 0:1], awsb[:, k:k + 1], WSA)
                            nc.vector.tensor_scalar_mul(
                                w2h8[:, k, 0:1], awsb[:, KU + k:KU + k + 1],
                                WSA)
                            nc.scalar.mul(aw3s[:, k:k + 1],
                                          awsb[:, 8 + k:9 + k], WSA)
                        nc.sync.dma_start(ab_sb, ab[None, :])
                        nc.scalar.mul(nab_sb, ab_sb, -1.0)
                        fb = stg.tile([1, U], F32, tag="fb")
                        nc.sync.dma_start(fb, ffb[None, :])
                        nc.vector.tensor_scalar_mul(ffb_h, fb, WSF)
                        fb2 = stg.tile([1, U], F32, tag="fb")
                        nc.sync.dma_start(fb2, frb[None, :])
                        nc.vector.tensor_scalar_mul(frb_h, fb2, WSF)
                    else:
                        emit_weights(1, 0)
                        emit_weights(1, 1)

            # ===== Phase B+C: highway stack; per-slab attention prep =======
            # wide [128,1024] 2-bank psum tiles; fp8 DoubleRow matmuls.
            # During layer 2, each finished 1024-token slab immediately gets
            # its row-major transposes, w3x, s1/thr and s2 emitted, filling
            # PE slack in the elementwise-bound highway stretch.
            with tc.tile_pool(name="hwp", bufs=2, space="PSUM") as hwp, \
                 tc.tile_pool(name="pcp", bufs=2, space="PSUM") as pcp, \
                 tc.tile_pool(name="pcp1", bufs=1, space="PSUM") as pcp1, \
                 tc.tile_pool(name="hws", bufs=3) as hws:
                s2p = pcp1.tile([128, NT], F32, tag="s2p")

                def prep_slab(tp):
                    """attention prep for tokens [tp*1024, (tp+1)*1024)."""
                    for k in range(KU):
                        wsl = slice(tp * 1024, (tp + 1) * 1024)
                        nc.vector.tensor_scalar_mul(
                            w3x8[:, k, wsl], x2q8[:, k, wsl],
                            aw3s[:, k:k + 1])
                    for jt in range(8 * tp, 8 * tp + 8):
                        # full-bank staging tile so rotating bufs land in
                        # different banks (PE-write vs DVE-read collision)
                        ptr = pcp.tile([128, 2048], F8, tag="ptr")
                        ptv = ptr[:, 0:1024].rearrange(
                            "p (n two) -> p n two", two=2)
                        for k in range(KU):
                            nc.tensor.transpose(
                                ptv[:, k * 128:(k + 1) * 128, 0:1],
                                x2q8[:, k, jt * 128:(jt + 1) * 128], ident8)
                        if jt % 2 == 0:
                            nc.vector.tensor_copy(xO8[:, jt, :],
                                                  ptv[:, :, 0:1])
                        else:
                            nc.scalar.copy(xO8[:, jt, :], ptv[:, :, 0:1])
                    for t in (2 * tp, 2 * tp + 1):
                        ps1 = pcp1.tile([1, 512], F32, tag="ps1")
                        for kk in range(KU // 2):
                            nc.tensor.matmul(
                                ps1, pair(w1h8, kk, slice(0, 1)),
                                pair(x2q8, kk,
                                     slice(t * 512, (t + 1) * 512)),
                                perf_mode=DR,
                                start=(kk == 0), stop=(kk == 1))
                        nc.scalar.activation(
                            thr[:, t * 512:(t + 1) * 512], ps1, AF.Exp,
                            bias=nab_sb, scale=-1.0 / WSA)
                    for jt in range(8 * tp, 8 * tp + 8):
                        jsl = slice(jt * 128, (jt + 1) * 128)
                        for kk in range(KU // 2):
                            nc.tensor.matmul(
                                s2p[:, jt:jt + 1],
                                pair(x2q8, kk, jsl),
                                pair(w2h8, kk, slice(0, 1)),
                                perf_mode=DR,
                                start=(kk == 0), stop=(kk == 1))
                    nc.scalar.mul(s2f[:, 8 * tp:8 * tp + 8],
                                  s2p[:, 8 * tp:8 * tp + 8], 1.0 / WSA)

                for l in range(H):
                    xin = x0q8 if l == 0 else x1q8
                    xout = x1q8 if l == 0 else x2q8
                    for t in range(NS):                # 512-token slabs
                        nsl = slice(t * 512, (t + 1) * 512)
                        for m in range(KU):
                            msl = slice(m * 128, (m + 1) * 128)
                            pt = hwp.tile([128, 512], F32, tag="pt")
                            pc = hwp.tile([128, 512], F32, tag="pc")
                            for kk in range(KU // 2):
                                nc.tensor.matmul(
                                    pt, pair(tWh8[:, l], kk, msl),
                                    pair(xin, kk, nsl), perf_mode=DR,
                                    start=(kk == 0), stop=(kk == 1))
                            for kk in range(KU // 2):
                                nc.tensor.matmul(
                                    pc, pair(cWh8[:, l], kk, msl),
                                    pair(xin, kk, nsl), perf_mode=DR,
                                    start=(kk == 0), stop=(kk == 1))
                            th = hws.tile([128, 512], BF16, tag="th")
                            ch = hws.tile([128, 512], BF16, tag="ch")
                            nc.scalar.activation(
                                th, pt, AF.Relu, bias=tbsb[:, l, m:m + 1],
                                scale=1.0 / WSH)
                            nc.scalar.activation(
                                ch, pc, AF.Sigmoid, bias=cbsb[:, l, m:m + 1],
                                scale=1.0 / WSH)
                            dh = hws.tile([128, 512], BF16, tag="dh")
                            nc.vector.tensor_tensor(
                                dh, th, xin[:, m, nsl], op=OP.subtract)
                            mh = hws.tile([128, 512], BF16, tag="mh")
                            nc.vector.tensor_tensor(
                                mh, ch, dh, op=OP.mult)
                            nc.gpsimd.tensor_tensor(
                                xout[:, m, nsl], xin[:, m, nsl], mh,
                                op=OP.add)
                        if l == H - 1 and t % 2 == 1:
                            prep_slab(t // 2)

            # ============= Phase D: pairwise softmax attention =============
            fWv = ffW.rearrange("(k p) m -> k p m", p=128)
            rWv = frW.rearrange("(k p) m -> k p m", p=128)
            # x-half (k 0..3) -> bf16 *256 ; att-half (k 4..7) -> fp8 *32
            fuse_chunks = ([(fWv, ffWx, ffW8, k) for k in range(2 * KU)] +
                           [(rWv, frWx, frW8, k) for k in range(2 * KU)])
            with tc.tile_pool(name="pdn", bufs=2, space="PSUM") as pdn, \
                 tc.tile_pool(name="pds", bufs=2, space="PSUM") as pds, \
                 tc.tile_pool(name="pdr", bufs=1, space="PSUM") as pdr, \
                 tc.tile_pool(name="pbc", bufs=1, space="PSUM") as pbc, \
                 tc.tile_pool(name="stgf", bufs=4) as stgf, \
                 tc.tile_pool(name="dsb", bufs=4) as dsb, \
                 tc.tile_pool(name="ehp", bufs=6) as ehp:
                ones2v = ones2c8.rearrange("p (two s) -> p two s", two=2)
                for b in range(BPC):
                    for h in range(IH):
                        # drip-feed fuse-gate weight loads (DMA idle here)
                        unit = b * IH + h
                        for ci in range(unit * 4, unit * 4 + 4):
                            wv_, wbf_, w8_, k_ = fuse_chunks[ci]
                            wsf = stgf.tile([128, U], F32, tag="wsf",
                                            name=f"wsf_{ci}")
                            nc.sync.dma_start(wsf, wv_[k_])
                            if k_ < KU:
                                if ci % 2 == 0:
                                    nc.vector.tensor_scalar_mul(
                                        wbf_[:, k_, :], wsf, WSF)
                                else:
                                    nc.scalar.mul(wbf_[:, k_, :], wsf, WSF)
                            else:
                                if ci % 2 == 0:
                                    nc.vector.tensor_scalar_mul(
                                        w8_[:, k_ - KU, :], wsf, WSH)
                                else:
                                    nc.scalar.mul(w8_[:, k_ - KU, :], wsf,
                                                  WSH)
                        isl = slice(b * L + h * 512, b * L + (h + 1) * 512)
                        thbc = dsb.tile([128, 512], BF16, tag="thbc")
                        pb1 = pbc.tile([128, 512], F32, tag="pb",
                                       name=f"pb1_{b}_{h}")
                        nc.tensor.matmul(pb1, ones_row, thr[:, isl],
                                         start=True, stop=True)
                        nc.scalar.copy(thbc, pb1)
                        # ---- stage 1: scores -> exp -> fp8 eh pair tiles
                        ehs = []
                        for p in range(JT // 2):      # j-tile pairs
                            jg = b * JT + 2 * p
                            ps = pds.tile([128, 1024], F32, tag="ps",
                                          name=f"ps_{b}_{h}_{p}")
                            ehb = ehp.tile([128, 1024], F8, tag="ehb",
                                           name=f"ehb_{b}_{h}_{p}")
                            ehbf = dsb.tile([128, 1024], BF16, tag="ehbf")
                            for half in range(2):
                                jsl = slice((jg + half) * 128,
                                            (jg + half + 1) * 128)
                                hsl = slice(half * 512, (half + 1) * 512)
                                for kk in range(KU // 2):
                                    nc.tensor.matmul(
                                        ps[:, hsl], pair(w3x8, kk, jsl),
                                        pair(x2q8, kk, isl), perf_mode=DR,
                                        start=(kk == 0), stop=(kk == 1))
                                nc.scalar.activation(
                                    ehbf[:, hsl], ps[:, hsl], AF.Exp,
                                    bias=s2f[:, jg + half:jg + half + 1],
                                    scale=1.0 / WSA)
                                nc.vector.tensor_tensor(
                                    ehb[:, hsl], ehbf[:, hsl], thbc,
                                    op=OP.max)
                            ehs.append(
                                ehb.rearrange("p (two n) -> p two n", two=2))
                        # ---- stage 2: denominator first, then du-major
                        # numerator accumulation
                        pr = pdr.tile([1, 512], F32, tag="pr")
                        for p in range(JT // 2):
                            nc.tensor.matmul(
                                pr, ones2v[:, :, 0:1], ehs[p], perf_mode=DR,
                                start=(p == 0), stop=(p == JT // 2 - 1))
                        rec = dsb.tile([1, 512], F32, tag="rec")
                        nc.vector.reciprocal_approx_fast(rec, pr)
                        rech = dsb.tile([1, 512], BF16, tag="rech")
                        nc.scalar.mul(rech, rec, ATS)
                        rbc = dsb.tile([128, 512], BF16, tag="rbc")
                        pb2 = pbc.tile([128, 512], F32, tag="pb",
                                       name=f"pb2_{b}_{h}")
                        nc.tensor.matmul(pb2, ones_row, rech,
                                         start=True, stop=True)
                        nc.scalar.copy(rbc, pb2)
                        for du in range(KU):
                            pn = pdn.tile([128, 512], F32, tag="pn",
                                          name=f"pn_{b}_{h}_{du}")
                            for p in range(JT // 2):
                                jg = b * JT + 2 * p
                                nc.tensor.matmul(
                                    pn,
                                    xO8[:, jg:jg + 2,
                                        du * 128:(du + 1) * 128],
                                    ehs[p], perf_mode=DR,
                                    start=(p == 0), stop=(p == JT // 2 - 1))
                            pnh = dsb.tile([128, 512], BF16, tag="pnh",
                                           name=f"pnh_{b}_{h}_{du}")
                            if du % 2 == 0:
                                nc.scalar.copy(pnh, pn)
                            else:
                                nc.vector.tensor_copy(pnh, pn)
                            nc.vector.tensor_tensor(
                                attT8[:, du, isl], pnh, rbc, op=OP.mult)

            # ============= Phase E: fuse gates + output ====================
            with tc.tile_pool(name="pep", bufs=2, space="PSUM") as pep, \
                 tc.tile_pool(name="esb", bufs=3) as esb:
                for mt in range(NT):
                    msl = slice(mt * 128, (mt + 1) * 128)
                    x0t = esb.tile([128, U], F32, tag="x0t")
                    nc.sync.dma_start(x0t, xv[mt])
                    pz = pep.tile([128, 512], F32, tag="pz")
                    pr2 = pep.tile([128, 512], F32, tag="pr2")
                    for k in range(KU):          # x-half, bf16
                        nc.tensor.matmul(pz, xTh[:, k, msl], ffWx[:, k, :],
                                         start=(k == 0), stop=False)
                        nc.tensor.matmul(pr2, xTh[:, k, msl], frWx[:, k, :],
                                         start=(k == 0), stop=False)
                    for kk in range(KU // 2):    # att-half, fp8 DR
                        nc.tensor.matmul(pz, pair(attT8, kk, msl),
                                         pair(ffW8, kk), perf_mode=DR,
                                         start=False, stop=False)
                        nc.tensor.matmul(pr2, pair(attT8, kk, msl),
                                         pair(frW8, kk), perf_mode=DR,
                                         start=False, stop=False)
                    nc.tensor.matmul(pz, ones_row, ffb_h, start=False,
                                     stop=True)
                    nc.tensor.matmul(pr2, ones_row, frb_h, start=False,
                                     stop=True)
                    zh = esb.tile([128, U], BF16, tag="zh")
                    rh = esb.tile([128, U], BF16, tag="rh")
                    q = esb.tile([128, U], F32, tag="q")
                    p2 = esb.tile([128, U], F32, tag="p2")
                    ot = esb.tile([128, U], F32, tag="ot")
                    if mt == NT - 1:
                        # shorten the kernel tail: split across engines
                        hU = U // 2
                        nc.scalar.activation(zh, pz, AF.Sigmoid,
                                             scale=1.0 / WSF)
                        nc.scalar.square(q, zh)
                        nc.scalar.activation(rh, pr2, AF.Sigmoid,
                                             scale=1.0 / WSF)
                        nc.vector.tensor_tensor(p2[:, :hU], rh[:, :hU],
                                                x0t[:, :hU], op=OP.mult)
                        nc.gpsimd.tensor_tensor(p2[:, hU:], rh[:, hU:],
                                                x0t[:, hU:], op=OP.mult)
                        nc.vector.tensor_tensor(ot[:, :hU], q[:, :hU],
                                                p2[:, :hU], op=OP.add)
                        nc.gpsimd.tensor_tensor(ot[:, hU:], q[:, hU:],
                                                p2[:, hU:], op=OP.add)
                    else:
                        nc.scalar.activation(zh, pz, AF.Sigmoid,
                                             scale=1.0 / WSF)
                        nc.scalar.activation(rh, pr2, AF.Sigmoid,
                                             scale=1.0 / WSF)
                        nc.scalar.square(q, zh)
                        nc.vector.tensor_tensor(p2, rh, x0t, op=OP.mult)
                        nc.vector.tensor_tensor(ot, q, p2, op=OP.add)
                    nc.sync.dma_start(outv[mt], ot)

    nc.compile()
    return nc


_NC_CACHE = None


def _get_nc():
    global _NC_CACHE
    if _NC_CACHE is None:
        _NC_CACHE = build_nc()
    return _NC_CACHE


def kernel(**inputs) -> np.ndarray:
    from concourse.bass_utils import run_bass_kernel_spmd

    nc = _get_nc()
    full = {k: np.ascontiguousarray(np.asarray(v, dtype=np.float32))
            for k, v in inputs.items()}
    in_maps = []
    for c in range(NCORES):
        m = dict(full)
        m["inputs"] = np.ascontiguousarray(
            full["inputs"][c * BPC:(c + 1) * BPC])
        in_maps.append(m)
    res = run_bass_kernel_spmd(nc, in_maps, core_ids=list(range(NCORES)))
    return np.concatenate([res.results[c]["out"] for c in range(NCORES)],
                          axis=0)


# revision 16
# speedup vs baseline: 1.1441x; 1.0221x over previous
"""Trainium2 Bass kernel for nn_Encoding_layer (highway stack + pairwise MLP
attention + fuse gates).

Sharding: data-parallel over batch B=16 across 8 NeuronCores (2 batches per
core); all dense weights replicated. No collectives.

v3: fp8-e4m3 DoubleRow matmuls for the compute-heavy GEMMs, with the
schedule restructured to keep the PE HAM clock-gate warm:
  - DoubleRow contracts 256 rows/pass (2 fp8 weights per PE cell); operand
    pairs are adjacent k-tiles in the free dim of the [128, KU, N] tilings.
  - Quantization (numpy-validated, rel err ~3e-3 vs 2e-2 budget):
      highway (x fp8, W fp8*32)   scores s3 (w3x fp8*64 x x2 fp8)
      att numerator (xO fp8 x eh fp8)   att stored fp8*8
      fuse gates: x-half bf16 (W bf16*256) + att-half fp8 DR (att*8 x W*32)
    All scales undone via scalar.activation(func, scale=2^-k).
  - Attention prep (row-major transposes, s1/s2/thr) is emitted per-slab
    inside highway layer 2, filling PE slack in the elementwise-bound
    highway stretch.
  - Phase D per (b,h) unit is two-staged: (1) all four j-tile-pair score
    blocks -> exp -> fp8 eh pair tiles (SBUF), (2) denominator then
    du-major numerator accumulation, so wide score psum is double-buffered
    within the 8-bank budget.
  - eh pair tiles [128,2,512] fp8 are exactly the DoubleRow moving operand
    of the numerator.  relu-as-clamp: M^T = max(exp(s3+s2), exp(-(s1+ab)))
    (the per-column factor exp(s1+ab) cancels in the softmax).
"""

import numpy as np

B, L, U, H = 16, 1024, 512, 2
NCORES = 8
BPC = B // NCORES          # batches per core
N = BPC * L                # token columns per core
KU = U // 128              # 4  u-tiles
NT = N // 128              # 16 row-tiles per core
NS = N // 512              # 4  512-wide column slices per core
JT = L // 128              # 8  j-tiles per batch
IH = L // 512              # 2  i-halves per batch

WSH = 32.0                 # highway weight prescale (2^5)
WSA = 64.0                 # aW prescale (2^6)
WSF = 256.0                # fuse-gate effective prescale (2^8)
ATS = 8.0                  # att fp8 prescale (2^3)


def build_nc():
    import concourse.bacc as bacc
    import concourse.tile as tile
    from concourse import mybir
    from concourse.masks import make_identity

    F32 = mybir.dt.float32
    BF16 = mybir.dt.bfloat16
    F8 = mybir.dt.float8e4
    AF = mybir.ActivationFunctionType
    OP = mybir.AluOpType
    DR = mybir.MatmulPerfMode.DoubleRow

    nc = bacc.Bacc("TRN2", target_bir_lowering=False, debug=False,
                   num_devices=NCORES)

    x_in = nc.dram_tensor("inputs", [BPC, L, U], F32, kind="ExternalInput").ap()
    tW = nc.dram_tensor("tW", [H, U, U], F32, kind="ExternalInput").ap()
    tb = nc.dram_tensor("tb", [H, U], F32, kind="ExternalInput").ap()
    cW = nc.dram_tensor("cW", [H, U, U], F32, kind="ExternalInput").ap()
    cb = nc.dram_tensor("cb", [H, U], F32, kind="ExternalInput").ap()
    aW = nc.dram_tensor("aW", [3 * U], F32, kind="ExternalInput").ap()
    ab = nc.dram_tensor("ab", [1], F32, kind="ExternalInput").ap()
    frW = nc.dram_tensor("frW", [2 * U, U], F32, kind="ExternalInput").ap()
    frb = nc.dram_tensor("frb", [U], F32, kind="ExternalInput").ap()
    ffW = nc.dram_tensor("ffW", [2 * U, U], F32, kind="ExternalInput").ap()
    ffb = nc.dram_tensor("ffb", [U], F32, kind="ExternalInput").ap()
    out = nc.dram_tensor("out", [BPC, L, U], F32, kind="ExternalOutput").ap()

    xv = x_in.flatten_outer_dims().rearrange("(t p) u -> t p u", p=128)
    outv = out.flatten_outer_dims().rearrange("(t p) u -> t p u", p=128)

    def pair(t, k2, sl=None):
        """[128, 2, *] DoubleRow view of adjacent k-tiles k2*2, k2*2+1."""
        return t[:, 2 * k2:2 * k2 + 2, sl] if sl is not None \
            else t[:, 2 * k2:2 * k2 + 2, :]

    with tile.TileContext(nc) as tc:
        with tc.tile_pool(name="pers", bufs=1) as pers:
            # ---- persistent SBUF tensors ----
            x0row = pers.tile([128, NT, U], F32, tag="x0row")  # inputs row-maj
            xTh = pers.tile([128, KU, N], BF16, tag="xTh")     # inputs^T bf16
            x0q8 = pers.tile([128, KU, N], F8, tag="x0q8")     # inputs^T fp8
            x1q8 = pers.tile([128, KU, N], F8, tag="x1q8")
            x2q8 = pers.tile([128, KU, N], F8, tag="x2q8")
            w3x8 = pers.tile([128, KU, N], F8, tag="w3x8")     # (w3*64)*x2^T
            attT8 = pers.tile([128, KU, N], F8, tag="attT8")   # att^T * 8
            xO8 = pers.tile([128, NT, U], F8, tag="xO8")       # row-major x2
            tWh8 = pers.tile([128, H, KU, U], F8, tag="tWh8")  # *32
            cWh8 = pers.tile([128, H, KU, U], F8, tag="cWh8")  # *32
            ffWx = pers.tile([128, KU, U], BF16, tag="ffWx")   # x-half *256
            frWx = pers.tile([128, KU, U], BF16, tag="frWx")
            ffW8 = pers.tile([128, KU, U], F8, tag="ffW8")     # att-half *32
            frW8 = pers.tile([128, KU, U], F8, tag="frW8")
            tbsb = pers.tile([128, H, KU], F32, tag="tbsb")
            cbsb = pers.tile([128, H, KU], F32, tag="cbsb")
            awsb = pers.tile([128, 12], F32, tag="awsb")       # w1|w2|w3 cols
            w1h8 = pers.tile([128, KU, 16], F8, tag="w1h8")    # *64, col 0
            w2h8 = pers.tile([128, KU, 16], F8, tag="w2h8")    # *64, col 0
            aw3s = pers.tile([128, KU], F32, tag="aw3s")       # w3 * 64 f32
            ab_sb = pers.tile([1, 1], F32, tag="ab_sb")
            nab_sb = pers.tile([1, 1], F32, tag="nab_sb")
            ffb_h = pers.tile([1, U], BF16, tag="ffb_h")       # *256
            frb_h = pers.tile([1, U], BF16, tag="frb_h")       # *256
            thr = pers.tile([1, N], BF16, tag="thr")   # exp(-(s1+ab))
            s2f = pers.tile([128, NT], F32, tag="s2f")
            ones_row = pers.tile([1, 128], BF16, tag="ones_row")
            ones2c8 = pers.tile([128, 32], F8, tag="ones2c8")  # DR ones pairs
            identb = pers.tile([128, 128], BF16, tag="identb")
            ident8 = pers.tile([128, 128], F8, tag="ident8")
            identf = pers.tile([128, 128], F32, tag="identf")

            nc.vector.memset(ones_row, 1.0)
            nc.vector.memset(ones2c8, 1.0)
            make_identity(nc, identb)
            make_identity(nc, ident8)
            make_identity(nc, identf)

            # ================= Phase A: loads, casts, input transpose ======
            with tc.tile_pool(name="stg", bufs=8) as stg, \
                 tc.tile_pool(name="stgw", bufs=8) as stgw, \
                 tc.tile_pool(name="ptA", bufs=1, space="PSUM") as ptA:
                warmp = ptA.tile([128, 512], F32, tag="warmp")

                def keep_warm(n, who):
                    for i in range(n):
                        nc.tensor.matmul(warmp[:, 0:128], identb, identb,
                                         start=True, stop=True)

                # highway-weight loads interleaved after tg0/tg1 so layer-0
                # can start as soon as the first column group lands; one
                # 1MB DMA + one wide cast per (layer, gate)
                def emit_weights(l, wi):
                    wsrc, wdst = ((tW, tWh8), (cW, cWh8))[wi]
                    wv = wsrc[l].rearrange("(k p) m -> p k m", p=128)
                    ws = stgw.tile([128, KU, U], F32, tag="ws",
                                   name=f"ws_{l}_{wi}")
                    nc.sync.dma_start(ws, wv)
                    if wi == 0:
                        nc.vector.tensor_scalar_mul(wdst[:, l], ws, WSH)
                    else:
                        nc.scalar.mul(wdst[:, l], ws, WSH)

                # warm the PE HAM clock-gate during the initial DMA wait
                keep_warm(48, "init")
                for tg in range(NS):
                    # one 1MB DMA per 512-token group, straight into the
                    # persistent row-major copy (reused by phase E)
                    nc.sync.dma_start(
                        x0row[:, 4 * tg:4 * tg + 4, :],
                        x_in.flatten_outer_dims().rearrange(
                            "(t p) u -> p t u", p=128)[:, 4 * tg:4 * tg + 4])
                    ptk = [ptA.tile([128, 512], F32, tag=f"ptk{k}",
                                    name=f"ptk_{tg}_{k}")
                           for k in range(KU)]
                    for tt in range(4):
                        t = tg * 4 + tt
                        for k in range(KU):
                            nc.tensor.transpose(
                                ptk[k][:, tt * 128:(tt + 1) * 128],
                                x0row[:, t, k * 128:(k + 1) * 128], identf)
                    for k in range(KU):
                        sl = slice(tg * 512, (tg + 1) * 512)
                        if k % 2 == 0:
                            nc.vector.tensor_copy(xTh[:, k, sl], ptk[k])
                            nc.scalar.copy(x0q8[:, k, sl], ptk[k])
                        else:
                            nc.scalar.copy(xTh[:, k, sl], ptk[k])
                            nc.vector.tensor_copy(x0q8[:, k, sl], ptk[k])
                    keep_warm(10, f"tg{tg}")
                    if tg < H:
                        emit_weights(0, tg)
                    elif tg == H:
                        nc.sync.dma_start(
                            tbsb, tb.rearrange("l (m p) -> p l m", p=128))
                        nc.sync.dma_start(
                            cbsb, cb.rearrange("l (m p) -> p l m", p=128))
                        nc.sync.dma_start(
                            awsb, aW.rearrange("(w m p) -> p (w m)",
                                               p=128, w=3))
                        for k in range(KU):
                            nc.vector.tensor_scalar_mul(
                                w1h8[:, k, 0:1], awsb[:, k:k + 1], WSA)
                            nc.vector.tensor_scalar_mul(
                                w2h8[:, k, 0:1], awsb[:, KU + k:KU + k + 1],
                                WSA)
                            nc.scalar.mul(aw3s[:, k:k + 1],
                                          awsb[:, 8 + k:9 + k], WSA)
                        nc.sync.dma_start(ab_sb, ab[None, :])
                        nc.scalar.mul(nab_sb, ab_sb, -1.0)
                        fb = stg.tile([1, U], F32, tag="fb")
                        nc.sync.dma_start(fb, ffb[None, :])
                        nc.vector.tensor_scalar_mul(ffb_h, fb, WSF)
                        fb2 = stg.tile([1, U], F32, tag="fb")
                        nc.sync.dma_start(fb2, frb[None, :])
                        nc.vector.tensor_scalar_mul(frb_h, fb2, WSF)
                    else:
                        emit_weights(1, 0)
                        emit_weights(1, 1)

            # ===== Phase B layer 0: wide [128,1024] 2-bank psum tiles ======
            with tc.tile_pool(name="hw0", bufs=2, space="PSUM") as hw0, \
                 tc.tile_pool(name="hs0", bufs=3) as hs0:
                for tp in range(NS // 2):              # 1024-token slabs
                    wsl = slice(tp * 1024, (tp + 1) * 1024)
                    for m in range(KU):
                        msl = slice(m * 128, (m + 1) * 128)
                        pt = hw0.tile([128, 1024], F32, tag="pt")
                        pc = hw0.tile([128, 1024], F32, tag="pc")
                        for h2 in range(2):
                            nsl = slice(tp * 1024 + h2 * 512,
                                        tp * 1024 + (h2 + 1) * 512)
                            psl = slice(h2 * 512, (h2 + 1) * 512)
                            for kk in range(KU // 2):
                                nc.tensor.matmul(
                                    pt[:, psl], pair(tWh8[:, 0], kk, msl),
                                    pair(x0q8, kk, nsl), perf_mode=DR,
                                    start=(kk == 0), stop=(kk == 1))
                            for kk in range(KU // 2):
                                nc.tensor.matmul(
                                    pc[:, psl], pair(cWh8[:, 0], kk, msl),
                                    pair(x0q8, kk, nsl), perf_mode=DR,
                                    start=(kk == 0), stop=(kk == 1))
                        th = hs0.tile([128, 1024], BF16, tag="th")
                        ch = hs0.tile([128, 1024], BF16, tag="ch")
                        nc.scalar.activation(
                            th, pt, AF.Relu, bias=tbsb[:, 0, m:m + 1],
                            scale=1.0 / WSH)
                        nc.scalar.activation(
                            ch, pc, AF.Sigmoid, bias=cbsb[:, 0, m:m + 1],
                            scale=1.0 / WSH)
                        dh = hs0.tile([128, 1024], BF16, tag="dh")
                        nc.vector.tensor_tensor(
                            dh, th, x0q8[:, m, wsl], op=OP.subtract)
                        mh = hs0.tile([128, 1024], BF16, tag="mh")
                        nc.vector.tensor_tensor(mh, ch, dh, op=OP.mult)
                        nc.gpsimd.tensor_tensor(
                            x1q8[:, m, wsl], x0q8[:, m, wsl], mh, op=OP.add)

            # ===== Phase B layer 1 + C: highway + per-slab attention prep ==
            # During layer 1, each finished 1024-token slab immediately gets
            # its row-major transposes, w3x, s1/thr and s2 emitted, filling
            # PE slack in the elementwise-bound highway stretch.
            with tc.tile_pool(name="hwp", bufs=2, space="PSUM") as hwp, \
                 tc.tile_pool(name="pcp", bufs=2, space="PSUM") as pcp, \
                 tc.tile_pool(name="pcp1", bufs=1, space="PSUM") as pcp1, \
                 tc.tile_pool(name="hws", bufs=3) as hws:
                s2p = pcp1.tile([128, NT], F32, tag="s2p")

                def prep_slab(tp):
                    """attention prep for tokens [tp*1024, (tp+1)*1024)."""
                    for k in range(KU):
                        wsl = slice(tp * 1024, (tp + 1) * 1024)
                        nc.vector.tensor_scalar_mul(
                            w3x8[:, k, wsl], x2q8[:, k, wsl],
                            aw3s[:, k:k + 1])
                    for jt in range(8 * tp, 8 * tp + 8):
                        # full-bank staging tile so rotating bufs land in
                        # different banks (PE-write vs DVE-read collision)
                        ptr = pcp.tile([128, 2048], F8, tag="ptr")
                        ptv = ptr[:, 0:1024].rearrange(
                            "p (n two) -> p n two", two=2)
                        for k in range(KU):
                            nc.tensor.transpose(
                                ptv[:, k * 128:(k + 1) * 128, 0:1],
                                x2q8[:, k, jt * 128:(jt + 1) * 128], ident8)
                        if jt % 2 == 0:
                            nc.vector.tensor_copy(xO8[:, jt, :],
                                                  ptv[:, :, 0:1])
                        else:
                            nc.scalar.copy(xO8[:, jt, :], ptv[:, :, 0:1])
                    for t in (2 * tp, 2 * tp + 1):
                        ps1 = pcp1.tile([1, 512], F32, tag="ps1")
                        for kk in range(KU // 2):
                            nc.tensor.matmul(
                                ps1, pair(w1h8, kk, slice(0, 1)),
                                pair(x2q8, kk,
                                     slice(t * 512, (t + 1) * 512)),
                                perf_mode=DR,
                                start=(kk == 0), stop=(kk == 1))
                        nc.scalar.activation(
                            thr[:, t * 512:(t + 1) * 512], ps1, AF.Exp,
                            bias=nab_sb, scale=-1.0 / WSA)
                    for jt in range(8 * tp, 8 * tp + 8):
                        jsl = slice(jt * 128, (jt + 1) * 128)
                        for kk in range(KU // 2):
                            nc.tensor.matmul(
                                s2p[:, jt:jt + 1],
                                pair(x2q8, kk, jsl),
                                pair(w2h8, kk, slice(0, 1)),
                                perf_mode=DR,
                                start=(kk == 0), stop=(kk == 1))
                    nc.scalar.mul(s2f[:, 8 * tp:8 * tp + 8],
                                  s2p[:, 8 * tp:8 * tp + 8], 1.0 / WSA)

                for t in range(NS):                    # 512-token slabs
                    nsl = slice(t * 512, (t + 1) * 512)
                    for m in range(KU):
                        msl = slice(m * 128, (m + 1) * 128)
                        pt = hwp.tile([128, 512], F32, tag="pt")
                        pc = hwp.tile([128, 512], F32, tag="pc")
                        for kk in range(KU // 2):
                            nc.tensor.matmul(
                                pt, pair(tWh8[:, 1], kk, msl),
                                pair(x1q8, kk, nsl), perf_mode=DR,
                                start=(kk == 0), stop=(kk == 1))
                        for kk in range(KU // 2):
                            nc.tensor.matmul(
                                pc, pair(cWh8[:, 1], kk, msl),
                                pair(x1q8, kk, nsl), perf_mode=DR,
                                start=(kk == 0), stop=(kk == 1))
                        th = hws.tile([128, 512], BF16, tag="th")
                        ch = hws.tile([128, 512], BF16, tag="ch")
                        nc.scalar.activation(
                            th, pt, AF.Relu, bias=tbsb[:, 1, m:m + 1],
                            scale=1.0 / WSH)
                        nc.scalar.activation(
                            ch, pc, AF.Sigmoid, bias=cbsb[:, 1, m:m + 1],
                            scale=1.0 / WSH)
                        dh = hws.tile([128, 512], BF16, tag="dh")
                        nc.vector.tensor_tensor(
                            dh, th, x1q8[:, m, nsl], op=OP.subtract)
                        mh = hws.tile([128, 512], BF16, tag="mh")
                        nc.vector.tensor_tensor(
                            mh, ch, dh, op=OP.mult)
                        nc.gpsimd.tensor_tensor(
                            x2q8[:, m, nsl], x1q8[:, m, nsl], mh,
                            op=OP.add)
                    if t % 2 == 1:
                        prep_slab(t // 2)

            # ============= Phase D: pairwise softmax attention =============
            fWv = ffW.rearrange("(k p) m -> k p m", p=128)
            rWv = frW.rearrange("(k p) m -> k p m", p=128)
            # x-half (k 0..3) -> bf16 *256 ; att-half (k 4..7) -> fp8 *32
            fuse_chunks = ([(fWv, ffWx, ffW8, k) for k in range(2 * KU)] +
                           [(rWv, frWx, frW8, k) for k in range(2 * KU)])
            with tc.tile_pool(name="pdn", bufs=2, space="PSUM") as pdn, \
                 tc.tile_pool(name="pds", bufs=2, space="PSUM") as pds, \
                 tc.tile_pool(name="pdr", bufs=1, space="PSUM") as pdr, \
                 tc.tile_pool(name="pbc", bufs=1, space="PSUM") as pbc, \
                 tc.tile_pool(name="stgf", bufs=4) as stgf, \
                 tc.tile_pool(name="dsb", bufs=4) as dsb, \
                 tc.tile_pool(name="ehp", bufs=6) as ehp:
                ones2v = ones2c8.rearrange("p (two s) -> p two s", two=2)
                for b in range(BPC):
                    for h in range(IH):
                        # drip-feed fuse-gate weight loads (DMA idle here)
                        unit = b * IH + h
                        for ci in range(unit * 4, unit * 4 + 4):
                            wv_, wbf_, w8_, k_ = fuse_chunks[ci]
                            wsf = stgf.tile([128, U], F32, tag="wsf",
                                            name=f"wsf_{ci}")
                            nc.sync.dma_start(wsf, wv_[k_])
                            if k_ < KU:
                                if ci % 2 == 0:
                                    nc.vector.tensor_scalar_mul(
                                        wbf_[:, k_, :], wsf, WSF)
                                else:
                                    nc.scalar.mul(wbf_[:, k_, :], wsf, WSF)
                            else:
                                if ci % 2 == 0:
                                    nc.vector.tensor_scalar_mul(
                                        w8_[:, k_ - KU, :], wsf, WSH)
                                else:
                                    nc.scalar.mul(w8_[:, k_ - KU, :], wsf,
                                                  WSH)
                        isl = slice(b * L + h * 512, b * L + (h + 1) * 512)
                        thbc = dsb.tile([128, 512], BF16, tag="thbc")
                        pb1 = pbc.tile([128, 512], F32, tag="pb",
                                       name=f"pb1_{b}_{h}")
                        nc.tensor.matmul(pb1, ones_row, thr[:, isl],
                                         start=True, stop=True)
                        nc.scalar.copy(thbc, pb1)
                        # ---- stage 1: scores -> exp -> fp8 eh pair tiles
                        ehs = []
                        for p in range(JT // 2):      # j-tile pairs
                            jg = b * JT + 2 * p
                            ps = pds.tile([128, 1024], F32, tag="ps",
                                          name=f"ps_{b}_{h}_{p}")
                            ehb = ehp.tile([128, 1024], F8, tag="ehb",
                                           name=f"ehb_{b}_{h}_{p}")
                            ehbf = dsb.tile([128, 1024], BF16, tag="ehbf")
                            for half in range(2):
                                jsl = slice((jg + half) * 128,
                                            (jg + half + 1) * 128)
                                hsl = slice(half * 512, (half + 1) * 512)
                                for kk in range(KU // 2):
                                    nc.tensor.matmul(
                                        ps[:, hsl], pair(w3x8, kk, jsl),
                                        pair(x2q8, kk, isl), perf_mode=DR,
                                        start=(kk == 0), stop=(kk == 1))
                                nc.scalar.activation(
                                    ehbf[:, hsl], ps[:, hsl], AF.Exp,
                                    bias=s2f[:, jg + half:jg + half + 1],
                                    scale=1.0 / WSA)
                                nc.vector.tensor_tensor(
                                    ehb[:, hsl], ehbf[:, hsl], thbc,
                                    op=OP.max)
                            ehs.append(
                                ehb.rearrange("p (two n) -> p two n", two=2))
                        # ---- stage 2: denominator first, then du-major
                        # numerator accumulation
                        pr = pdr.tile([1, 512], F32, tag="pr")
                        for p in range(JT // 2):
                            nc.tensor.matmul(
                                pr, ones2v[:, :, 0:1], ehs[p], perf_mode=DR,
                                start=(p == 0), stop=(p == JT // 2 - 1))
                        rec = dsb.tile([1, 512], F32, tag="rec")
                        nc.vector.reciprocal_approx_fast(rec, pr)
                        rech = dsb.tile([1, 512], BF16, tag="rech")
                        nc.scalar.mul(rech, rec, ATS)
                        rbc = dsb.tile([128, 512], BF16, tag="rbc")
                        pb2 = pbc.tile([128, 512], F32, tag="pb",
                                       name=f"pb2_{b}_{h}")
                        nc.tensor.matmul(pb2, ones_row, rech,
                                         start=True, stop=True)
                        nc.scalar.copy(rbc, pb2)
                        for du in range(KU):
                            pn = pdn.tile([128, 512], F32, tag="pn",
                                          name=f"pn_{b}_{h}_{du}")
                            for p in range(JT // 2):
                                jg = b * JT + 2 * p
                                nc.tensor.matmul(
                                    pn,
                                    xO8[:, jg:jg + 2,
                                        du * 128:(du + 1) * 128],
                                    ehs[p], perf_mode=DR,
                                    start=(p == 0), stop=(p == JT // 2 - 1))
                            # drain + normalize (*8) in one pass
                            nc.vector.tensor_tensor(
                                attT8[:, du, isl], pn, rbc, op=OP.mult)

            # ============= Phase E: fuse gates + output ====================
            with tc.tile_pool(name="pep", bufs=2, space="PSUM") as pep, \
                 tc.tile_pool(name="peb", bufs=1, space="PSUM") as peb, \
                 tc.tile_pool(name="esb", bufs=3) as esb:
                # broadcast fuse biases (*256) to [128, 512] once
                fbb = esb.tile([128, U], BF16, tag="fbb")
                rbb = esb.tile([128, U], BF16, tag="rbb")
                pfb = peb.tile([128, 512], F32, tag="pfb", name="pfb_f")
                nc.tensor.matmul(pfb, ones_row, ffb_h, start=True, stop=True)
                nc.vector.tensor_copy(fbb, pfb)
                prb = peb.tile([128, 512], F32, tag="pfb", name="pfb_r")
                nc.tensor.matmul(prb, ones_row, frb_h, start=True, stop=True)
                nc.vector.tensor_copy(rbb, prb)
                for mt in range(NT):
                    msl = slice(mt * 128, (mt + 1) * 128)
                    pz = pep.tile([128, 512], F32, tag="pz")
                    pr2 = pep.tile([128, 512], F32, tag="pr2")
                    for k in range(KU):          # x-half, bf16
                        nc.tensor.matmul(pz, xTh[:, k, msl], ffWx[:, k, :],
                                         start=(k == 0), stop=False)
                        nc.tensor.matmul(pr2, xTh[:, k, msl], frWx[:, k, :],
                                         start=(k == 0), stop=False)
                    for kk in range(KU // 2):    # att-half, fp8 DR
                        nc.tensor.matmul(pz, pair(attT8, kk, msl),
                                         pair(ffW8, kk), perf_mode=DR,
                                         start=False, stop=(kk == 1))
                        nc.tensor.matmul(pr2, pair(attT8, kk, msl),
                                         pair(frW8, kk), perf_mode=DR,
                                         start=False, stop=(kk == 1))
                    # bias add on vector (frees psum early), sigmoid on
                    # scalar from SBUF
                    pzs = esb.tile([128, U], BF16, tag="pzs")
                    prs = esb.tile([128, U], BF16, tag="prs")
                    nc.vector.tensor_tensor(pzs, pz, fbb, op=OP.add)
                    nc.vector.tensor_tensor(prs, pr2, rbb, op=OP.add)
                    zh = esb.tile([128, U], BF16, tag="zh")
                    rh = esb.tile([128, U], BF16, tag="rh")
                    q = esb.tile([128, U], F32, tag="q")
                    p2 = esb.tile([128, U], F32, tag="p2")
                    ot = esb.tile([128, U], F32, tag="ot")
                    x0t = x0row[:, mt, :]
                    if mt == NT - 1:
                        # shorten the kernel tail: split across engines
                        hU = U // 2
                        nc.scalar.activation(zh, pzs, AF.Sigmoid,
                                             scale=1.0 / WSF)
                        nc.scalar.square(q, zh)
                        nc.scalar.activation(rh, prs, AF.Sigmoid,
                                             scale=1.0 / WSF)
                        nc.vector.tensor_tensor(p2[:, :hU], rh[:, :hU],
                                                x0t[:, :hU], op=OP.mult)
                        nc.gpsimd.tensor_tensor(p2[:, hU:], rh[:, hU:],
                                                x0t[:, hU:], op=OP.mult)
                        nc.vector.tensor_tensor(ot[:, :hU], q[:, :hU],
                                                p2[:, :hU], op=OP.add)
                        nc.gpsimd.tensor_tensor(ot[:, hU:], q[:, hU:],
                                                p2[:, hU:], op=OP.add)
                    else:
                        nc.scalar.activation(zh, pzs, AF.Sigmoid,
                                             scale=1.0 / WSF)
                        nc.scalar.activation(rh, prs, AF.Sigmoid,
                                             scale=1.0 / WSF)
                        nc.scalar.square(q, zh)
                        nc.vector.tensor_tensor(p2, rh, x0t, op=OP.mult)
                        nc.vector.tensor_tensor(ot, q, p2, op=OP.add)
                    nc.sync.dma_start(outv[mt], ot)

    nc.compile()
    return nc


_NC_CACHE = None


def _get_nc():
    global _NC_CACHE
    if _NC_CACHE is None:
        _NC_CACHE = build_nc()
    return _NC_CACHE


def kernel(**inputs) -> np.ndarray:
    from concourse.bass_utils import run_bass_kernel_spmd

    nc = _get_nc()
    full = {k: np.ascontiguousarray(np.asarray(v, dtype=np.float32))
            for k, v in inputs.items()}
    in_maps = []
    for c in range(NCORES):
        m = dict(full)
        m["inputs"] = np.ascontiguousarray(
            full["inputs"][c * BPC:(c + 1) * BPC])
        in_maps.append(m)
    res = run_bass_kernel_spmd(nc, in_maps, core_ids=list(range(NCORES)))
    return np.concatenate([res.results[c]["out"] for c in range(NCORES)],
                          axis=0)
